# revision 1
# baseline (speedup 1.0000x reference)
"""Ragged cross-attention pooling kernel for Trainium2 (8 NeuronCores, SPMD).

Math (per pair, direction "A attends over B"):
    qa = (A @ Wq + bq) * scale          [la, INNER]
    kb =  B @ Wk + bk                   [lb, INNER]
    s  = qa @ kb^T  (+ -1e9 on pad k)   [la, lb]
    p  = exp(s)               (no max-subtraction needed: |s| <~ 6)
    den[q] = sum_k p[q, k]
    g[q] = valid(q) / (la * den[q])
    w[k] = sum_q g[q] p[q, k]           <- collapses the mean over queries
    emb  = (w^T B) @ Wv + bv            <- collapses attn@V and the V projection

Distribution: 64 pairs -> 8 slots x 8 cores (one shared SPMD program, shapes
fixed per slot to the max over cores; pairs bin-packed by length so padding is
small).  All matmuls run as float32r (~13-bit mantissa, 1 cyc/row).
"""

import os
import sys

sys.path.insert(0, "/opt/trn_rl_repo")

import numpy as np

B, LA, LB, DIM, INNER, OUTER = 64, 1024, 1024, 640, 256, 1024
NCORES, NSLOTS, P = 8, 8, 128
SCALE = 1.0 / np.sqrt(INNER)
NEGM = -1.0e9
DT = DIM // P  # 5 d-chunks

LAST_EXEC_TIME_NS = None

# N-chunk splits (multiples of 128 up to 1024) keeping chunks <=512 and >=256
# where possible (float32r runs 4x slower below N=256).
_CHUNKS = {
    128: [128], 256: [256], 384: [384], 512: [512],
    640: [512, 128], 768: [512, 256], 896: [512, 384], 1024: [512, 512],
}


def _chunks(total):
    out, off = [], 0
    for c in _CHUNKS[total]:
        out.append((off, c))
        off += c
    return out


def _plan(la_all, lb_all):
    """Assign pairs to (slot, core); returns swap flags, groups, slot tile shapes."""
    la = np.asarray(la_all, np.int64)
    lb = np.asarray(lb_all, np.int64)
    swap = lb > la
    qa = np.where(swap, lb, la)  # kernel A-side length (>= B-side)
    qb = np.where(swap, la, lb)
    at = -(-qa // P)
    bt = -(-qb // P)
    order = np.argsort(-(at * 1024 + bt), kind="stable")
    groups = [list(order[s * NCORES:(s + 1) * NCORES]) for s in range(NSLOTS)]
    C1, C2 = 3200.0, 420.0

    def gcost(g):
        ma = max(at[i] for i in g)
        mb = max(bt[i] for i in g)
        return C1 * (ma + mb) + C2 * ma * mb

    rng = np.random.default_rng(0)
    cost = [gcost(g) for g in groups]
    s1s = rng.integers(0, NSLOTS, 30000)
    s2s = rng.integers(0, NSLOTS, 30000)
    i1s = rng.integers(0, NCORES, 30000)
    i2s = rng.integers(0, NCORES, 30000)
    for s1, s2, i1, i2 in zip(s1s, s2s, i1s, i2s):
        if s1 == s2:
            continue
        g1 = groups[s1][:]
        g2 = groups[s2][:]
        g1[i1], g2[i2] = groups[s2][i2], groups[s1][i1]
        n1, n2 = gcost(g1), gcost(g2)
        if n1 + n2 < cost[s1] + cost[s2] - 1e-9:
            groups[s1], groups[s2] = g1, g2
            cost[s1], cost[s2] = n1, n2
    slot_at = [max(at[i] for i in g) for g in groups]
    slot_bt = [max(bt[i] for i in g) for g in groups]
    # reserve >=1 pad column on each key side (den-correction needs one)
    for s in range(NSLOTS):
        if max(qa[i] for i in groups[s]) == slot_at[s] * P:
            slot_at[s] += 1
        if max(qb[i] for i in groups[s]) == slot_bt[s] * P:
            slot_bt[s] += 1
    return swap, qa, qb, groups, slot_at, slot_bt


def _build_program(slot_at, slot_bt):
    import concourse.bass as bass  # noqa: F401
    import concourse.mybir as mybir
    import concourse.tile as tile
    from concourse import bacc

    F32 = mybir.dt.float32
    F32R = mybir.dt.float32r
    BF16 = mybir.dt.bfloat16
    Exp = mybir.ActivationFunctionType.Exp
    Ident = mybir.ActivationFunctionType.Identity

    tot_at = sum(slot_at)
    tot_bt = sum(slot_bt)
    cum_at = np.concatenate([[0], np.cumsum(slot_at)]).astype(int)
    cum_bt = np.concatenate([[0], np.cumsum(slot_bt)]).astype(int)

    nc = bacc.Bacc("TRN2", target_bir_lowering=False, debug=False,
                   num_devices=NCORES)

    abuf = nc.dram_tensor("abuf", [tot_at * P, DIM], F32R, kind="ExternalInput")
    bbuf = nc.dram_tensor("bbuf", [tot_bt * P, DIM], F32R, kind="ExternalInput")
    gs_a_d = nc.dram_tensor("gs_a", [P, tot_at], F32, kind="ExternalInput")
    gs_b_d = nc.dram_tensor("gs_b", [P, tot_bt], F32, kind="ExternalInput")
    npa_d = nc.dram_tensor("npa", [P, NSLOTS], F32, kind="ExternalInput")
    npb_d = nc.dram_tensor("npb", [P, NSLOTS], F32, kind="ExternalInput")
    wq_d = nc.dram_tensor("wq", [P, DT, INNER], BF16, kind="ExternalInput")
    wk_d = nc.dram_tensor("wk", [P, DT, INNER], BF16, kind="ExternalInput")
    wv_d = nc.dram_tensor("wv", [P, DT, OUTER], F32R, kind="ExternalInput")
    bqs_d = nc.dram_tensor("bqs", [P, INNER // P], F32, kind="ExternalInput")
    bk_d = nc.dram_tensor("bk", [P, INNER // P], F32, kind="ExternalInput")
    bv_d = nc.dram_tensor("bv", [P, OUTER // P], F32, kind="ExternalInput")
    id_d = nc.dram_tensor("ident", [P, P], F32R, kind="ExternalInput")
    idb_d = nc.dram_tensor("identb", [P, P], BF16, kind="ExternalInput")
    emb_d = nc.dram_tensor("emb", [P, OUTER // P, 2 * NSLOTS], F32,
                           kind="ExternalOutput")
    DBG = os.environ.get("KBDBG", "0") == "1"
    if DBG:
        d_at = nc.dram_tensor("d_at", [P, DT, slot_at[0] * P], F32,
                              kind="ExternalOutput")
        d_qaT = nc.dram_tensor("d_qaT", [P, INNER // P, slot_at[0] * P], F32,
                               kind="ExternalOutput")
        d_kbT = nc.dram_tensor("d_kbT", [P, INNER // P, slot_bt[0] * P], F32,
                               kind="ExternalOutput")
        d_p = nc.dram_tensor("d_p", [P, slot_bt[0] * P], F32,
                             kind="ExternalOutput")
        d_wrow = nc.dram_tensor("d_wrow", [1, slot_bt[0] * P], F32,
                                kind="ExternalOutput")
        d_wcol = nc.dram_tensor("d_wcol", [P, slot_bt[0]], F32,
                                kind="ExternalOutput")
        d_ur = nc.dram_tensor("d_ur", [1, DIM], F32, kind="ExternalOutput")
        d_urows = nc.dram_tensor("d_urows", [2 * NSLOTS, DIM], F32,
                                 kind="ExternalOutput")
        d_usb = nc.dram_tensor("d_usb", [P, DT, 2 * NSLOTS], F32,
                               kind="ExternalOutput")
        d_wrow16 = nc.dram_tensor("d_wrow16", [2 * NSLOTS, 1024], F32,
                                  kind="ExternalOutput")
        d_p16 = nc.dram_tensor("d_p16", [2 * NSLOTS, P, 1024], F32,
                               kind="ExternalOutput")
        d_den16 = nc.dram_tensor("d_den16", [2 * NSLOTS, P, 1], F32,
                                 kind="ExternalOutput")
        d_gcol16 = nc.dram_tensor("d_gcol16", [2 * NSLOTS, P, 1], F32,
                                  kind="ExternalOutput")

    with tile.TileContext(nc) as tc:
        with (
            tc.tile_pool(name="const", bufs=1) as cpool,
            tc.tile_pool(name="anat", bufs=2) as apool,
            tc.tile_pool(name="bnat", bufs=2) as bpool,
            tc.tile_pool(name="att", bufs=1) as atpool,
            tc.tile_pool(name="proj", bufs=1) as ppool,
            tc.tile_pool(name="pexp", bufs=2) as epool,
            tc.tile_pool(name="small", bufs=2) as spool,
            tc.tile_pool(name="km", bufs=2) as kmpool,
            tc.tile_pool(name="late", bufs=1) as lpool,
            tc.tile_pool(name="psA", bufs=4, space="PSUM") as psA,
            tc.tile_pool(name="psW", bufs=1, space="PSUM") as psW,
            tc.tile_pool(name="psU", bufs=1, space="PSUM") as psU,
        ):
            # ---- constants ----
            wq_sb = cpool.tile([P, DT, INNER], BF16, tag="wq")
            wk_sb = cpool.tile([P, DT, INNER], BF16, tag="wk")
            bqs_sb = cpool.tile([P, INNER // P], F32, tag="bqs")
            bk_sb = cpool.tile([P, INNER // P], F32, tag="bk")
            bv_sb = cpool.tile([P, OUTER // P], F32, tag="bv")
            id_sb = cpool.tile([P, P], F32R, tag="ident")
            idb_sb = cpool.tile([P, P], BF16, tag="identb")
            npa_sb = cpool.tile([P, NSLOTS], F32, tag="npa")
            npb_sb = cpool.tile([P, NSLOTS], F32, tag="npb")
            gs_a_sb = cpool.tile([P, tot_at], F32, tag="gsa")
            gs_b_sb = cpool.tile([P, tot_bt], F32, tag="gsb")
            urows_sb = cpool.tile([2 * NSLOTS, DIM], F32R, tag="urows")
            for sb, d in ((wq_sb, wq_d), (wk_sb, wk_d),
                          (bqs_sb, bqs_d), (bk_sb, bk_d), (bv_sb, bv_d),
                          (id_sb, id_d), (idb_sb, idb_d),
                          (npa_sb, npa_d), (npb_sb, npb_d),
                          (gs_a_sb, gs_a_d), (gs_b_sb, gs_b_d)):
                nc.sync.dma_start(sb[:], d[:])

            for s in range(NSLOTS):
                at_s, bt_s = int(slot_at[s]), int(slot_bt[s])
                pla, plb = at_s * P, bt_s * P
                # ---- load A/B (natural layout, row-tiled) ----
                anat = apool.tile([P, at_s, DIM], F32R, tag="anat")
                bnat = bpool.tile([P, bt_s, DIM], F32R, tag="bnat")
                nc.sync.dma_start(
                    anat[:], abuf[cum_at[s] * P:(cum_at[s] + at_s) * P, :]
                    .rearrange("(t p) d -> p t d", p=P))
                nc.sync.dma_start(
                    bnat[:], bbuf[cum_bt[s] * P:(cum_bt[s] + bt_s) * P, :]
                    .rearrange("(t p) d -> p t d", p=P))

                # ---- transposes: AT[pd, dt, q], BT[pd, dt, k] ----
                at_sb = atpool.tile([P, DT, pla], BF16, tag="at")
                bt_sb = atpool.tile([P, DT, plb], BF16, tag="bt")
                cp = 0
                for nat, tsb, nt in ((anat, at_sb, at_s),
                                     (bnat, bt_sb, bt_s)):
                    for dt in range(DT):
                        bfc = kmpool.tile([P, nt, P], BF16, tag="bfc",
                                          name="bfc")
                        nc.vector.tensor_copy(
                            bfc[:], nat[:, :, dt * P:(dt + 1) * P])
                        for q0 in range(0, nt, 4):
                            qn = min(4, nt - q0)
                            tp = psA.tile([P, 512], F32, tag="mmps")
                            for j in range(qn):
                                nc.tensor.matmul(
                                    tp[:, j * P:(j + 1) * P],
                                    bfc[:, q0 + j, :], idb_sb[:],
                                    start=True, stop=True)
                            dst = tsb[:, dt, q0 * P:(q0 + qn) * P]
                            if cp % 2 == 0:
                                nc.vector.tensor_copy(dst, tp[:, :qn * P])
                            else:
                                nc.scalar.copy(dst, tp[:, :qn * P])
                            cp += 1

                # ---- projections: qaT, kaT from AT; qbT, kbT from BT ----
                qaT = ppool.tile([P, INNER // P, pla], F32R, tag="qaT")
                kaT = ppool.tile([P, INNER // P, pla], F32R, tag="kaT")
                qbT = ppool.tile([P, INNER // P, plb], F32R, tag="qbT")
                kbT = ppool.tile([P, INNER // P, plb], F32R, tag="kbT")
                for dst, src, pl, w_sb, bias, scl in (
                        (qaT, at_sb, pla, wq_sb, bqs_sb, SCALE),
                        (kaT, at_sb, pla, wk_sb, bk_sb, 1.0),
                        (qbT, bt_sb, plb, wq_sb, bqs_sb, SCALE),
                        (kbT, bt_sb, plb, wk_sb, bk_sb, 1.0)):
                    for m in range(INNER // P):
                        for noff, nlen in _chunks(pl):
                            pp = psA.tile([P, 512], F32, tag="mmps")
                            for kt in range(DT):
                                nc.tensor.matmul(
                                    pp[:, :nlen],
                                    w_sb[:, kt, m * P:(m + 1) * P],
                                    src[:, kt, noff:noff + nlen],
                                    start=(kt == 0), stop=(kt == DT - 1))
                            if m == 0:
                                nc.vector.tensor_scalar(
                                    dst[:, m, noff:noff + nlen], pp[:, :nlen],
                                    scl, bias[:, m, None],
                                    mybir.AluOpType.mult, mybir.AluOpType.add)
                            else:
                                nc.scalar.activation(
                                    dst[:, m, noff:noff + nlen], pp[:, :nlen],
                                    Ident, bias=bias[:, m, None], scale=scl)

                if DBG and s == 0:
                    nc.gpsimd.dma_start(d_at[:], at_sb[:])
                    nc.sync.dma_start(d_qaT[:], qaT[:].bitcast(F32))
                    nc.sync.dma_start(d_kbT[:], kbT[:].bitcast(F32))

                # ---- attention directions ----
                for dr in range(2):
                    if dr == 0:  # A queries over B keys
                        QT, KT, nq, nk = qaT, kbT, at_s, bt_s
                        g_sb, g_off = gs_a_sb, cum_at[s]
                        np_sb = npb_sb
                        knat = bnat
                    else:
                        QT, KT, nq, nk = qbT, kaT, bt_s, at_s
                        g_sb, g_off = gs_b_sb, cum_bt[s]
                        np_sb = npa_sb
                        knat = anat
                    plk = nk * P
                    kchunks = _chunks(plk)
                    wr = [psW.tile([1, cl], F32, tag=f"wr{ci}",
                                   name=f"wr{ci}")
                          for ci, (co, cl) in enumerate(kchunks)]
                    for qt in range(nq):
                        scs, dens = [], []
                        den = spool.tile([P, 2], F32, tag="den")
                        p_sb = epool.tile([P, plk], F32R, tag="p_sb")
                        for ci, (co, cl) in enumerate(kchunks):
                            sc = psA.tile([P, 512], F32, tag="mmps")
                            for ki in range(INNER // P):
                                nc.tensor.matmul(
                                    sc[:, :cl],
                                    QT[:, ki, qt * P:(qt + 1) * P],
                                    KT[:, ki, co:co + cl],
                                    start=(ki == 0), stop=(ki == 1))
                            nc.scalar.activation(
                                p_sb[:, co:co + cl], sc[:, :cl], Exp,
                                bias=0.0, scale=1.0,
                                accum_out=den[:, ci:ci + 1])
                        if DBG and s == 0 and dr == 0 and qt == 0:
                            nc.sync.dma_start(d_p[:, :plk], p_sb[:].bitcast(F32))
                        if DBG and qt == 0:
                            nc.sync.dma_start(d_p16[2 * s + dr, :, :plk],
                                              p_sb[:].bitcast(F32))
                        rec = spool.tile([P, 1], F32, tag="rec")
                        if len(kchunks) == 2:
                            nc.vector.tensor_add(den[:, 0:1], den[:, 0:1],
                                                 den[:, 1:2])
                        # subtract the pad-column contribution exactly:
                        # all pad columns share the value p[:, plk-1]
                        pc = spool.tile([P, 1], F32, tag="pc")
                        nc.vector.tensor_mul(pc[:],
                                             p_sb[:, plk - 1:plk].bitcast(F32),
                                             np_sb[:, s:s + 1])
                        nc.vector.tensor_tensor(den[:, 0:1], den[:, 0:1],
                                                pc[:],
                                                mybir.AluOpType.subtract)
                        nc.vector.reciprocal(rec[:], den[:, 0:1])
                        gcol = spool.tile([P, 1], F32R, tag="gcol")
                        nc.vector.tensor_mul(gcol[:], rec[:],
                                             g_sb[:, g_off + qt, None])
                        if DBG and qt == 0:
                            nc.sync.dma_start(d_den16[2 * s + dr], den[:, 0:1])
                            nc.sync.dma_start(d_gcol16[2 * s + dr],
                                              gcol[:].bitcast(F32))
                        for ci, (co, cl) in enumerate(kchunks):
                            nc.tensor.matmul(
                                wr[ci][:], gcol[:], p_sb[:, co:co + cl],
                                start=(qt == 0), stop=(qt == nq - 1))
                    # w row -> w col (transpose via identity matmul)
                    wrow = lpool.tile([1, plk], F32R, tag="wrow")
                    for ci, (co, cl) in enumerate(kchunks):
                        nc.scalar.copy(wrow[0:1, co:co + cl], wr[ci][:])
                    wt = psA.tile([P, 2 * nk], F32, tag="mmps")
                    for kt in range(nk):
                        nc.tensor.matmul(
                            wt[:, 2 * kt:2 * kt + 2],
                            wrow[0:1, kt * P:(kt + 1) * P],
                            id_sb[0:1, 0:2], start=True, stop=True)
                    wcol = spool.tile([P, nk], F32R, tag="wcol")
                    nc.vector.tensor_copy(
                        wcol[:],
                        wt[:].rearrange("p (k two) -> p k two", two=2)[:, :, 0])
                    # u row = w^T @ Knat
                    ur = psU.tile([1, DIM], F32, tag="ur")
                    for noff, nlen in _chunks(DIM):
                        for kt in range(nk):
                            nc.tensor.matmul(
                                ur[0:1, noff:noff + nlen],
                                wcol[:, kt:kt + 1],
                                knat[:, kt, noff:noff + nlen],
                                start=(kt == 0), stop=(kt == nk - 1))
                    if DBG:
                        nc.sync.dma_start(d_wrow16[2 * s + dr, None, :plk],
                                          wrow[:].bitcast(F32))
                    if DBG and s == 0 and dr == 0:
                        nc.sync.dma_start(d_wrow[0:1, :plk], wrow[:].bitcast(F32))
                        nc.sync.dma_start(d_wcol[:, :nk], wcol[:].bitcast(F32))
                    ursb = lpool.tile([1, DIM], F32R, tag="ursb")
                    nc.scalar.copy(ursb[:], ur[:])
                    if DBG and s == 0 and dr == 0:
                        nc.sync.dma_start(d_ur[:], ursb[:].bitcast(F32))
                    nc.sync.dma_start(urows_sb[2 * s + dr:2 * s + dr + 1, :],
                                      ursb[:])

            # ---- final: E = Wv^T U + bv ----
            wv_sb = lpool.tile([P, DT, OUTER], F32R, tag="wv")
            nc.sync.dma_start(wv_sb[:], wv_d[:])
            u_sb = cpool.tile([P, DT, 2 * NSLOTS], F32R, tag="usb")
            for dt in range(DT):
                ut = psA.tile([P, 2 * NSLOTS], F32, tag="mmps")
                nc.tensor.matmul(
                    ut[:, :2 * NSLOTS],
                    urows_sb[:, dt * P:(dt + 1) * P],
                    id_sb[0:2 * NSLOTS, 0:2 * NSLOTS],
                    start=True, stop=True)
                nc.vector.tensor_copy(u_sb[:, dt, :], ut[:, :2 * NSLOTS])
            if DBG:
                nc.sync.dma_start(d_urows[:], urows_sb[:].bitcast(F32))
                nc.sync.dma_start(d_usb[:], u_sb[:].bitcast(F32))
            e_sb = cpool.tile([P, OUTER // P, 2 * NSLOTS], F32, tag="esb")
            for oc in range(OUTER // P):
                ep = psA.tile([P, 2 * NSLOTS], F32, tag="mmps")
                for dt in range(DT):
                    nc.tensor.matmul(
                        ep[:, :2 * NSLOTS],
                        wv_sb[:, dt, oc * P:(oc + 1) * P],
                        u_sb[:, dt, :],
                        start=(dt == 0), stop=(dt == DT - 1))
                nc.scalar.activation(e_sb[:, oc, :], ep[:, :2 * NSLOTS],
                                     Ident, bias=bv_sb[:, oc, None], scale=1.0)
            nc.sync.dma_start(emb_d[:], e_sb[:])

    nc.compile()
    return nc


def _install_profhook():
    import contextlib
    import ctypes
    import types

    import antenv

    if not hasattr(antenv, "axon_hooks"):
        mod = types.ModuleType("antenv.axon_hooks")
        mod._hook = None

        def _set(h):
            mod._hook = h

        def _get():
            return mod._hook

        mod.set_axon_ntff_profile_hook = _set
        mod.get_axon_ntff_profile_hook = _get
        sys.modules["antenv.axon_hooks"] = mod
        antenv.axon_hooks = mod
    from antenv.axon_hooks import set_axon_ntff_profile_hook
    so_path = "/opt/axon/libaxon_pjrt.so"
    if not os.path.exists(so_path):
        return False
    lib = ctypes.CDLL(so_path)
    if not hasattr(lib, "axon_start_nrt_profile"):
        return False
    lib.axon_start_nrt_profile.argtypes = [ctypes.POINTER(ctypes.c_int64),
                                           ctypes.c_size_t]
    lib.axon_start_nrt_profile.restype = ctypes.c_int64
    lib.axon_stop_nrt_profile.argtypes = [ctypes.c_char_p]
    lib.axon_stop_nrt_profile.restype = ctypes.c_int64

    @contextlib.contextmanager
    def _hook(output_dir, device_ids):
        import jax

        jax.devices()
        if device_ids:
            ids = (ctypes.c_int64 * len(device_ids))(*device_ids)
            rc = lib.axon_start_nrt_profile(ids, len(device_ids))
        else:
            rc = lib.axon_start_nrt_profile(None, 0)
        if rc != 0:
            raise RuntimeError(f"axon_start_nrt_profile rc={rc}")
        try:
            yield
        finally:
            n = lib.axon_stop_nrt_profile(str(output_dir).encode())
            print(f"profile: {n} file(s) written to {output_dir}",
                  file=sys.stderr)

    set_axon_ntff_profile_hook(_hook)
    return True


def kernel(a_pad, b_pad, len_a, len_b, Wq, bq, Wk, bk, Wv, bv):
    global LAST_EXEC_TIME_NS
    a_pad = np.ascontiguousarray(np.asarray(a_pad, np.float32))
    b_pad = np.ascontiguousarray(np.asarray(b_pad, np.float32))
    len_a = np.asarray(len_a, np.int32)
    len_b = np.asarray(len_b, np.int32)
    Wq = np.asarray(Wq, np.float32)
    Wk = np.asarray(Wk, np.float32)
    Wv = np.asarray(Wv, np.float32)
    bq = np.asarray(bq, np.float32)
    bk = np.asarray(bk, np.float32)
    bv = np.asarray(bv, np.float32)

    swap, qa_len, qb_len, groups, slot_at, slot_bt = _plan(len_a, len_b)
    tot_at, tot_bt = sum(slot_at), sum(slot_bt)
    cum_at = np.concatenate([[0], np.cumsum(slot_at)]).astype(int)
    cum_bt = np.concatenate([[0], np.cumsum(slot_bt)]).astype(int)

    # ---- shared (per-core-identical) inputs ----
    import ml_dtypes
    wq_h = Wq.reshape(DT, P, INNER).transpose(1, 0, 2).astype(ml_dtypes.bfloat16)
    wk_h = Wk.reshape(DT, P, INNER).transpose(1, 0, 2).astype(ml_dtypes.bfloat16)
    wv_h = Wv.reshape(DT, P, OUTER).transpose(1, 0, 2).copy()
    bqs_h = (bq * SCALE).reshape(INNER // P, P).T.copy()
    bk_h = bk.reshape(INNER // P, P).T.copy()
    bv_h = bv.reshape(OUTER // P, P).T.copy()
    id_h = np.eye(P, dtype=np.float32)
    idb_h = np.eye(P, dtype=np.float32).astype(ml_dtypes.bfloat16)

    # ---- per-core inputs ----
    in_maps = []
    for c in range(NCORES):
        abuf = np.zeros((tot_at * P, DIM), np.float32)
        bbuf = np.zeros((tot_bt * P, DIM), np.float32)
        gs_a = np.zeros((P, tot_at), np.float32)
        gs_b = np.zeros((P, tot_bt), np.float32)
        npa = np.zeros((P, NSLOTS), np.float32)
        npb = np.zeros((P, NSLOTS), np.float32)
        for s in range(NSLOTS):
            i = groups[s][c]
            la_i, lb_i = int(qa_len[i]), int(qb_len[i])
            A = b_pad[i] if swap[i] else a_pad[i]
            Bm = a_pad[i] if swap[i] else b_pad[i]
            abuf[cum_at[s] * P:cum_at[s] * P + la_i] = A[:la_i]
            bbuf[cum_bt[s] * P:cum_bt[s] * P + lb_i] = Bm[:lb_i]
            ga = np.zeros(slot_at[s] * P, np.float32)
            ga[:la_i] = 1.0 / la_i
            gs_a[:, cum_at[s]:cum_at[s] + slot_at[s]] = \
                ga.reshape(slot_at[s], P).T
            gb = np.zeros(slot_bt[s] * P, np.float32)
            gb[:lb_i] = 1.0 / lb_i
            gs_b[:, cum_bt[s]:cum_bt[s] + slot_bt[s]] = \
                gb.reshape(slot_bt[s], P).T
            npa[:, s] = slot_at[s] * P - la_i
            npb[:, s] = slot_bt[s] * P - lb_i
        in_maps.append({
            "abuf": abuf, "bbuf": bbuf, "gs_a": gs_a, "gs_b": gs_b,
            "npa": npa, "npb": npb, "wq": wq_h, "wk": wk_h, "wv": wv_h,
            "bqs": bqs_h, "bk": bk_h, "bv": bv_h, "ident": id_h,
            "identb": idb_h,
        })

    nc = _build_program(slot_at, slot_bt)

    from concourse.bass_utils import run_bass_kernel_spmd

    trace = os.environ.get("BASS_KERNEL_TRACE", "0") == "1"
    if trace:
        _install_profhook()
    res = run_bass_kernel_spmd(nc, in_maps, list(range(NCORES)), trace=trace)
    LAST_EXEC_TIME_NS = res.exec_time_ns

    emb_a = np.zeros((B, OUTER), np.float32)
    emb_b = np.zeros((B, OUTER), np.float32)
    for c in range(NCORES):
        e = res.results[c]["emb"].transpose(1, 0, 2).reshape(OUTER,
                                                            2 * NSLOTS)
        for s in range(NSLOTS):
            i = groups[s][c]
            ea, eb = e[:, 2 * s], e[:, 2 * s + 1]  # A-queries, B-queries
            if swap[i]:
                emb_a[i], emb_b[i] = eb, ea
            else:
                emb_a[i], emb_b[i] = ea, eb
    return emb_a, emb_b



# revision 3
# speedup vs baseline: 1.3991x; 1.3991x over previous
"""Ragged cross-attention pooling kernel for Trainium2 (8 NeuronCores, SPMD).

Math (per pair, direction "A attends over B"):
    qa = (A @ Wq + bq) * scale          [la, INNER]
    kb =  B @ Wk                        [lb, INNER]   (bk dropped: softmax
                                                       is shift-invariant per query)
    s  = qa @ kb^T                      [la, lb]      (pad k-cols are exactly 0)
    p  = exp(s)                                       (pad cols: exp(0) = 1.0)
    den[q] = sum_k p[q, k] - n_pad                    (exact pad correction)
    g[q] = valid(q) / (la * den[q])
    w[k] = sum_q g[q] p[q, k]           <- collapses the mean over queries
    emb  = (w^T B) @ Wv + bv            <- collapses attn@V and the V projection

Distribution: 64 pairs -> 8 slots x 8 cores (one shared SPMD program, shapes
fixed per slot to the max over cores; pairs bin-packed by length so padding is
small).

Perf notes vs the first version:
  - A/B uploaded BOTH pre-transposed (DIM-major, fp8e4m3, DIM zero-padded to
    768) for the Q/K path AND natural-layout bf16 for the value path; no
    on-device transposes at all.
  - Projections and QK^T run as fp8 DoubleRow matmuls (2 contraction rows per
    partition, 0.5 cyc/row).  INNER=256 = 2x128 maps exactly onto the
    DoubleRow pair dim for the scores.
  - Value path (w^T B, Wv^T u) in bf16 (1 cyc/row, no small-N penalty).
  - exp() is one activation per q-tile over a [128, plk] PSUM span with a
    single accumulator read for den.
"""

import os
import sys

sys.path.insert(0, "/opt/trn_rl_repo")

import numpy as np

B, LA, LB, DIM, INNER, OUTER = 64, 1024, 1024, 640, 256, 1024
NCORES, NSLOTS, P = 8, 8, 128
SCALE = 1.0 / np.sqrt(INNER)
DT = DIM // P       # 5 d-chunks of 128
DJ = 3              # DoubleRow d-pair chunks (768 = 3 * 256)
DPAD = DJ * 2 * P   # 768

LAST_EXEC_TIME_NS = None


def _chunks(total, step=512):
    out, off = [], 0
    while off < total:
        c = min(step, total - off)
        out.append((off, c))
        off += c
    return out


def _plan(la_all, lb_all):
    """Assign pairs to (slot, core); returns swap flags, groups, slot tile shapes."""
    la = np.asarray(la_all, np.int64)
    lb = np.asarray(lb_all, np.int64)
    swap = lb > la
    qa = np.where(swap, lb, la)  # kernel A-side length (>= B-side)
    qb = np.where(swap, la, lb)
    at = -(-qa // P)
    bt = -(-qb // P)
    order = np.argsort(-(at * 1024 + bt), kind="stable")
    groups = [list(order[s * NCORES:(s + 1) * NCORES]) for s in range(NSLOTS)]
    C1, C2 = 2000.0, 200.0

    def gcost(g):
        ma = max(at[i] for i in g)
        mb = max(bt[i] for i in g)
        return C1 * (ma + mb) + C2 * ma * mb

    rng = np.random.default_rng(0)
    cost = [gcost(g) for g in groups]
    s1s = rng.integers(0, NSLOTS, 30000)
    s2s = rng.integers(0, NSLOTS, 30000)
    i1s = rng.integers(0, NCORES, 30000)
    i2s = rng.integers(0, NCORES, 30000)
    for s1, s2, i1, i2 in zip(s1s, s2s, i1s, i2s):
        if s1 == s2:
            continue
        g1 = groups[s1][:]
        g2 = groups[s2][:]
        g1[i1], g2[i2] = groups[s2][i2], groups[s1][i1]
        n1, n2 = gcost(g1), gcost(g2)
        if n1 + n2 < cost[s1] + cost[s2] - 1e-9:
            groups[s1], groups[s2] = g1, g2
            cost[s1], cost[s2] = n1, n2
    slot_at = [int(max(at[i] for i in g)) for g in groups]
    slot_bt = [int(max(bt[i] for i in g)) for g in groups]
    return swap, qa, qb, groups, slot_at, slot_bt


def _build_program(slot_at, slot_bt):
    import concourse.bass as bass  # noqa: F401
    import concourse.mybir as mybir
    import concourse.tile as tile
    from concourse import bacc

    F32 = mybir.dt.float32
    F32R = mybir.dt.float32r
    BF16 = mybir.dt.bfloat16
    FP8 = mybir.dt.float8e4
    Exp = mybir.ActivationFunctionType.Exp
    Ident = mybir.ActivationFunctionType.Identity
    DR = mybir.MatmulPerfMode.DoubleRow
    Alu = mybir.AluOpType

    tot_at = sum(slot_at)
    tot_bt = sum(slot_bt)
    cum_at = np.concatenate([[0], np.cumsum(slot_at)]).astype(int)
    cum_bt = np.concatenate([[0], np.cumsum(slot_bt)]).astype(int)

    nc = bacc.Bacc("TRN2", target_bir_lowering=False, debug=False,
                   num_devices=NCORES)

    # transposed fp8 inputs: [p, j, i, tok] = X[tok, j*256 + i*128 + p]
    at8_d = nc.dram_tensor("at8", [P, DJ, 2, tot_at * P], FP8,
                           kind="ExternalInput")
    bt8_d = nc.dram_tensor("bt8", [P, DJ, 2, tot_bt * P], FP8,
                           kind="ExternalInput")
    # natural bf16 inputs: [p, T, d] = X[T*128 + p, d]
    an_d = nc.dram_tensor("an16", [P, tot_at, DIM], BF16, kind="ExternalInput")
    bn_d = nc.dram_tensor("bn16", [P, tot_bt, DIM], BF16, kind="ExternalInput")
    gs_a_d = nc.dram_tensor("gs_a", [P, tot_at], F32, kind="ExternalInput")
    gs_b_d = nc.dram_tensor("gs_b", [P, tot_bt], F32, kind="ExternalInput")
    npa_d = nc.dram_tensor("npa", [P, NSLOTS], F32, kind="ExternalInput")
    npb_d = nc.dram_tensor("npb", [P, NSLOTS], F32, kind="ExternalInput")
    wq_d = nc.dram_tensor("wq8", [P, DJ, 2, INNER], FP8, kind="ExternalInput")
    wk_d = nc.dram_tensor("wk8", [P, DJ, 2, INNER], FP8, kind="ExternalInput")
    wv_d = nc.dram_tensor("wv16", [P, DT, OUTER], BF16, kind="ExternalInput")
    bqs_d = nc.dram_tensor("bqs", [P, INNER // P], F32, kind="ExternalInput")
    bv_d = nc.dram_tensor("bv", [P, OUTER // P], F32, kind="ExternalInput")
    idr_d = nc.dram_tensor("idr", [P, P], F32R, kind="ExternalInput")
    idb_d = nc.dram_tensor("idb", [P, P], BF16, kind="ExternalInput")
    emb_d = nc.dram_tensor("emb", [P, OUTER // P, 2 * NSLOTS], F32,
                           kind="ExternalOutput")

    with tile.TileContext(nc) as tc:
        with (
            tc.tile_pool(name="const", bufs=1) as cpool,
            tc.tile_pool(name="ain", bufs=2) as apool,
            tc.tile_pool(name="proj", bufs=2) as ppool,
            tc.tile_pool(name="pexp", bufs=3) as epool,
            tc.tile_pool(name="small", bufs=3) as spool,
            tc.tile_pool(name="late", bufs=2) as lpool,
            tc.tile_pool(name="psA", bufs=2, space="PSUM") as psA,
            tc.tile_pool(name="psW", bufs=2, space="PSUM") as psW,
        ):
            # ---- constants ----
            wq_sb = cpool.tile([P, DJ, 2, INNER], FP8, tag="wq")
            wk_sb = cpool.tile([P, DJ, 2, INNER], FP8, tag="wk")
            bqs_sb = cpool.tile([P, INNER // P], F32, tag="bqs")
            bv_sb = cpool.tile([P, OUTER // P], F32, tag="bv")
            idr_sb = cpool.tile([P, P], F32R, tag="idr")
            idb_sb = cpool.tile([P, P], BF16, tag="idb")
            npa_sb = cpool.tile([P, NSLOTS], F32, tag="npa")
            npb_sb = cpool.tile([P, NSLOTS], F32, tag="npb")
            gs_a_sb = cpool.tile([P, tot_at], F32, tag="gsa")
            gs_b_sb = cpool.tile([P, tot_bt], F32, tag="gsb")
            wv_sb = cpool.tile([P, DT, OUTER], BF16, tag="wv")
            urows_sb = cpool.tile([2 * NSLOTS, DIM], BF16, tag="urows")
            for sb, d in ((wq_sb, wq_d), (wk_sb, wk_d), (bqs_sb, bqs_d),
                          (bv_sb, bv_d), (idr_sb, idr_d), (idb_sb, idb_d),
                          (npa_sb, npa_d), (npb_sb, npb_d),
                          (gs_a_sb, gs_a_d), (gs_b_sb, gs_b_d),
                          (wv_sb, wv_d)):
                nc.sync.dma_start(sb[:], d[:])

            for s in range(NSLOTS):
                at_s, bt_s = int(slot_at[s]), int(slot_bt[s])
                pla, plb = at_s * P, bt_s * P
                # ---- load A/B ----
                a8 = apool.tile([P, DJ, 2, pla], FP8, tag="a8")
                b8 = apool.tile([P, DJ, 2, plb], FP8, tag="b8")
                an = apool.tile([P, at_s, DIM], BF16, tag="an")
                bn = apool.tile([P, bt_s, DIM], BF16, tag="bn")
                nc.sync.dma_start(
                    a8[:], at8_d[:, :, :, cum_at[s] * P:(cum_at[s] + at_s) * P])
                nc.sync.dma_start(
                    b8[:], bt8_d[:, :, :, cum_bt[s] * P:(cum_bt[s] + bt_s) * P])
                nc.sync.dma_start(
                    an[:], an_d[:, cum_at[s]:cum_at[s] + at_s, :])
                nc.sync.dma_start(
                    bn[:], bn_d[:, cum_bt[s]:cum_bt[s] + bt_s, :])

                # ---- projections (fp8 DoubleRow): qT/kT [p, m, tok] ----
                qaT = ppool.tile([P, 2, pla], FP8, tag="qaT")
                kaT = ppool.tile([P, 2, pla], FP8, tag="kaT")
                qbT = ppool.tile([P, 2, plb], FP8, tag="qbT")
                kbT = ppool.tile([P, 2, plb], FP8, tag="kbT")
                for src, pl, dst, w_sb, is_q in (
                        (a8, pla, qaT, wq_sb, True),
                        (a8, pla, kaT, wk_sb, False),
                        (b8, plb, qbT, wq_sb, True),
                        (b8, plb, kbT, wk_sb, False)):
                    for m in range(2):
                        pp = psA.tile([P, 1024], F32, tag="mm")
                        for j in range(DJ):
                            for co, cl in _chunks(pl):
                                nc.tensor.matmul(
                                    pp[:, co:co + cl],
                                    w_sb[:, j, :, m * P:(m + 1) * P],
                                    src[:, j, :, co:co + cl],
                                    start=(j == 0), stop=(j == DJ - 1),
                                    perf_mode=DR)
                        if is_q:
                            # q = psum * scale + bq*scale   (fp8 out)
                            nc.vector.tensor_scalar(
                                dst[:, m, :], pp[:, :pl],
                                SCALE, bqs_sb[:, m, None],
                                Alu.mult, Alu.add)
                        else:
                            # k = psum (no bias; softmax shift-invariance)
                            nc.scalar.copy(dst[:, m, :], pp[:, :pl])

                # ---- attention directions ----
                for dr in range(2):
                    if dr == 0:  # A queries over B keys
                        QT, KT, nq, nk = qaT, kbT, at_s, bt_s
                        g_sb, g_off = gs_a_sb, cum_at[s]
                        np_sb = npb_sb
                        knat = bn
                    else:
                        QT, KT, nq, nk = qbT, kaT, bt_s, at_s
                        g_sb, g_off = gs_b_sb, cum_bt[s]
                        np_sb = npa_sb
                        knat = an
                    plk = nk * P
                    kch = _chunks(plk)
                    wr = psW.tile([1, 1024], F32, tag="wr")
                    den2 = None
                    p_tiles = {}
                    for qt in range(nq):
                        sc = psA.tile([P, 1024], F32, tag="mm")
                        for co, cl in kch:
                            nc.tensor.matmul(
                                sc[:, co:co + cl],
                                QT[:, :, qt * P:(qt + 1) * P],
                                KT[:, :, co:co + cl],
                                start=True, stop=True, perf_mode=DR)
                        if qt % 2 == 0:
                            den2 = spool.tile([P, 2], F32, tag="den")
                        p_sb = epool.tile([P, 1024], F32R, tag="p")
                        p_tiles[qt] = p_sb
                        nc.scalar.activation(
                            p_sb[:, :plk], sc[:, :plk], Exp,
                            accum_out=den2[:, qt % 2:qt % 2 + 1])
                        if qt % 2 == 1 or qt == nq - 1:
                            q0 = qt - (qt % 2)
                            npair = qt - q0 + 1
                            dpair = den2[:, :npair]
                            # den -= pad count (pad cols are exactly exp(0)=1)
                            nc.vector.tensor_scalar_sub(
                                dpair, dpair, np_sb[:, s:s + 1])
                            rec2 = spool.tile([P, 2], F32, tag="rec")
                            nc.vector.reciprocal(rec2[:, :npair], dpair)
                            gcol2 = spool.tile([P, 2], F32R, tag="gc")
                            nc.vector.tensor_tensor(
                                gcol2[:, :npair], rec2[:, :npair],
                                g_sb[:, g_off + q0:g_off + q0 + npair],
                                Alu.mult)
                            for qp in range(q0, qt + 1):
                                pt = p_tiles.pop(qp)
                                for co, cl in kch:
                                    nc.tensor.matmul(
                                        wr[0:1, co:co + cl],
                                        gcol2[:, qp - q0:qp - q0 + 1],
                                        pt[:, co:co + cl],
                                        start=(qp == 0), stop=(qp == nq - 1))
                    # w row -> w col (transpose via identity matmul)
                    wrow = lpool.tile([1, 1024], F32R, tag="wrow")
                    if dr == 0:
                        nc.scalar.copy(wrow[0:1, :plk], wr[0:1, :plk])
                    else:
                        nc.vector.tensor_copy(wrow[0:1, :plk], wr[0:1, :plk])
                    wt = psA.tile([P, 1024], F32, tag="mm")
                    for kt in range(nk):
                        nc.tensor.matmul(
                            wt[:, 2 * kt:2 * kt + 2],
                            wrow[0:1, kt * P:(kt + 1) * P],
                            idr_sb[0:1, 0:2], start=True, stop=True)
                    wcol = spool.tile([P, 8], BF16, tag="wcol")
                    nc.vector.tensor_copy(
                        wcol[:, :nk],
                        wt[:, :2 * nk].rearrange(
                            "p (k two) -> p k two", two=2)[:, :, 0])
                    # u row = w^T @ Knat   (bf16)
                    ur = psW.tile([1, 1024], F32, tag="wr")
                    for co, cl in _chunks(DIM):
                        for kt in range(nk):
                            nc.tensor.matmul(
                                ur[0:1, co:co + cl],
                                wcol[:, kt:kt + 1],
                                knat[:, kt, co:co + cl],
                                start=(kt == 0), stop=(kt == nk - 1))
                    ursb = lpool.tile([1, DIM], BF16, tag="ursb")
                    if dr == 0:
                        nc.vector.tensor_copy(ursb[:], ur[0:1, :DIM])
                    else:
                        nc.scalar.copy(ursb[:], ur[0:1, :DIM])
                    nc.sync.dma_start(urows_sb[2 * s + dr:2 * s + dr + 1, :],
                                      ursb[:])

            # ---- final: E = Wv^T U + bv ----
            u_sb = cpool.tile([P, DT, 2 * NSLOTS], BF16, tag="usb")
            for dt in range(DT):
                ut = psA.tile([P, 1024], F32, tag="mm")
                nc.tensor.matmul(
                    ut[:, :2 * NSLOTS],
                    urows_sb[:, dt * P:(dt + 1) * P],
                    idb_sb[0:2 * NSLOTS, 0:2 * NSLOTS],
                    start=True, stop=True)
                nc.vector.tensor_copy(u_sb[:, dt, :], ut[:, :2 * NSLOTS])
            e_sb = cpool.tile([P, OUTER // P, 2 * NSLOTS], F32, tag="esb")
            for oc in range(OUTER // P):
                ep = psA.tile([P, 1024], F32, tag="mm")
                for dt in range(DT):
                    nc.tensor.matmul(
                        ep[:, :2 * NSLOTS],
                        wv_sb[:, dt, oc * P:(oc + 1) * P],
                        u_sb[:, dt, :],
                        start=(dt == 0), stop=(dt == DT - 1))
                nc.scalar.activation(e_sb[:, oc, :], ep[:, :2 * NSLOTS],
                                     Ident, bias=bv_sb[:, oc, None], scale=1.0)
            nc.sync.dma_start(emb_d[:], e_sb[:])

    nc.compile()
    return nc


def _install_profhook():
    import contextlib
    import ctypes
    import types

    import antenv

    if not hasattr(antenv, "axon_hooks"):
        mod = types.ModuleType("antenv.axon_hooks")
        mod._hook = None

        def _set(h):
            mod._hook = h

        def _get():
            return mod._hook

        mod.set_axon_ntff_profile_hook = _set
        mod.get_axon_ntff_profile_hook = _get
        sys.modules["antenv.axon_hooks"] = mod
        antenv.axon_hooks = mod
    from antenv.axon_hooks import set_axon_ntff_profile_hook
    so_path = "/opt/axon/libaxon_pjrt.so"
    if not os.path.exists(so_path):
        return False
    lib = ctypes.CDLL(so_path)
    if not hasattr(lib, "axon_start_nrt_profile"):
        return False
    lib.axon_start_nrt_profile.argtypes = [ctypes.POINTER(ctypes.c_int64),
                                           ctypes.c_size_t]
    lib.axon_start_nrt_profile.restype = ctypes.c_int64
    lib.axon_stop_nrt_profile.argtypes = [ctypes.c_char_p]
    lib.axon_stop_nrt_profile.restype = ctypes.c_int64

    @contextlib.contextmanager
    def _hook(output_dir, device_ids):
        import jax

        jax.devices()
        if device_ids:
            ids = (ctypes.c_int64 * len(device_ids))(*device_ids)
            rc = lib.axon_start_nrt_profile(ids, len(device_ids))
        else:
            rc = lib.axon_start_nrt_profile(None, 0)
        if rc != 0:
            raise RuntimeError(f"axon_start_nrt_profile rc={rc}")
        try:
            yield
        finally:
            n = lib.axon_stop_nrt_profile(str(output_dir).encode())
            print(f"profile: {n} file(s) written to {output_dir}",
                  file=sys.stderr)

    set_axon_ntff_profile_hook(_hook)
    return True


def kernel(a_pad, b_pad, len_a, len_b, Wq, bq, Wk, bk, Wv, bv):
    global LAST_EXEC_TIME_NS
    import ml_dtypes
    FP8 = ml_dtypes.float8_e4m3fn
    BF16 = ml_dtypes.bfloat16

    a_pad = np.ascontiguousarray(np.asarray(a_pad, np.float32))
    b_pad = np.ascontiguousarray(np.asarray(b_pad, np.float32))
    len_a = np.asarray(len_a, np.int32)
    len_b = np.asarray(len_b, np.int32)
    Wq = np.asarray(Wq, np.float32)
    Wk = np.asarray(Wk, np.float32)
    Wv = np.asarray(Wv, np.float32)
    bq = np.asarray(bq, np.float32)
    bv = np.asarray(bv, np.float32)

    swap, qa_len, qb_len, groups, slot_at, slot_bt = _plan(len_a, len_b)
    tot_at, tot_bt = sum(slot_at), sum(slot_bt)
    cum_at = np.concatenate([[0], np.cumsum(slot_at)]).astype(int)
    cum_bt = np.concatenate([[0], np.cumsum(slot_bt)]).astype(int)

    # ---- shared (per-core-identical) inputs ----
    def pack_w8(W):
        # [640, INNER] -> [128, 3, 2, INNER] with d = j*256 + i*128 + p
        Wp = np.zeros((DPAD, W.shape[1]), np.float32)
        Wp[:DIM] = W
        return np.ascontiguousarray(
            Wp.reshape(DJ, 2, P, W.shape[1]).transpose(2, 0, 1, 3)
        ).astype(FP8)

    wq8 = pack_w8(Wq)
    wk8 = pack_w8(Wk)
    wv16 = np.ascontiguousarray(
        Wv.reshape(DT, P, OUTER).transpose(1, 0, 2)).astype(BF16)
    bqs_h = (bq * SCALE).reshape(INNER // P, P).T.copy()
    bv_h = bv.reshape(OUTER // P, P).T.copy()
    idr_h = np.eye(P, dtype=np.float32)
    idb_h = np.eye(P, dtype=np.float32).astype(BF16)

    # ---- per-core inputs ----
    in_maps = []
    for c in range(NCORES):
        abuf = np.zeros((tot_at * P, DPAD), np.float32)
        bbuf = np.zeros((tot_bt * P, DPAD), np.float32)
        gs_a = np.zeros((P, tot_at), np.float32)
        gs_b = np.zeros((P, tot_bt), np.float32)
        npa = np.zeros((P, NSLOTS), np.float32)
        npb = np.zeros((P, NSLOTS), np.float32)
        for s in range(NSLOTS):
            i = groups[s][c]
            la_i, lb_i = int(qa_len[i]), int(qb_len[i])
            A = b_pad[i] if swap[i] else a_pad[i]
            Bm = a_pad[i] if swap[i] else b_pad[i]
            abuf[cum_at[s] * P:cum_at[s] * P + la_i, :DIM] = A[:la_i]
            bbuf[cum_bt[s] * P:cum_bt[s] * P + lb_i, :DIM] = Bm[:lb_i]
            ga = np.zeros(slot_at[s] * P, np.float32)
            ga[:la_i] = 1.0 / la_i
            gs_a[:, cum_at[s]:cum_at[s] + slot_at[s]] = \
                ga.reshape(slot_at[s], P).T
            gb = np.zeros(slot_bt[s] * P, np.float32)
            gb[:lb_i] = 1.0 / lb_i
            gs_b[:, cum_bt[s]:cum_bt[s] + slot_bt[s]] = \
                gb.reshape(slot_bt[s], P).T
            npa[:, s] = slot_at[s] * P - la_i
            npb[:, s] = slot_bt[s] * P - lb_i
        # transposed fp8: [tok, 768] -> [128, 3, 2, tok]
        at8 = np.ascontiguousarray(
            abuf.reshape(tot_at * P, DJ, 2, P).transpose(3, 1, 2, 0)
        ).astype(FP8)
        bt8 = np.ascontiguousarray(
            bbuf.reshape(tot_bt * P, DJ, 2, P).transpose(3, 1, 2, 0)
        ).astype(FP8)
        # natural bf16: [tok, 640] -> [128, T, 640]
        an16 = np.ascontiguousarray(
            abuf[:, :DIM].reshape(tot_at, P, DIM).transpose(1, 0, 2)
        ).astype(BF16)
        bn16 = np.ascontiguousarray(
            bbuf[:, :DIM].reshape(tot_bt, P, DIM).transpose(1, 0, 2)
        ).astype(BF16)
        in_maps.append({
            "at8": at8, "bt8": bt8, "an16": an16, "bn16": bn16,
            "gs_a": gs_a, "gs_b": gs_b, "npa": npa, "npb": npb,
            "wq8": wq8, "wk8": wk8, "wv16": wv16,
            "bqs": bqs_h, "bv": bv_h, "idr": idr_h, "idb": idb_h,
        })

    nc = _build_program(slot_at, slot_bt)

    from concourse.bass_utils import run_bass_kernel_spmd

    trace = os.environ.get("BASS_KERNEL_TRACE", "0") == "1"
    if trace:
        _install_profhook()
    res = run_bass_kernel_spmd(nc, in_maps, list(range(NCORES)), trace=trace)
    LAST_EXEC_TIME_NS = res.exec_time_ns

    emb_a = np.zeros((B, OUTER), np.float32)
    emb_b = np.zeros((B, OUTER), np.float32)
    for c in range(NCORES):
        e = res.results[c]["emb"].transpose(1, 0, 2).reshape(OUTER,
                                                            2 * NSLOTS)
        for s in range(NSLOTS):
            i = groups[s][c]
            ea, eb = e[:, 2 * s], e[:, 2 * s + 1]  # A-queries, B-queries
            if swap[i]:
                emb_a[i], emb_b[i] = eb, ea
            else:
                emb_a[i], emb_b[i] = ea, eb
    return emb_a, emb_b


# revision 7
# speedup vs baseline: 1.7666x; 1.2627x over previous
"""Ragged cross-attention pooling kernel for Trainium2 (8 NeuronCores, SPMD).

Math (per pair, direction "A attends over B"):
    qa = (A @ Wq + bq) * scale          [la, INNER]
    kb =  B @ Wk                        [lb, INNER]   (bk dropped: softmax
                                                       is shift-invariant per query)
    s  = qa @ kb^T                      [la, lb]      (pad k-cols are exactly 0)
    p  = exp(s)                                       (pad cols: exp(0) = 1.0)
    den[q] = sum_k p[q, k] - n_pad                    (exact pad correction)
    g[q] = valid(q) / (la * den[q])
    w[k] = sum_q g[q] p[q, k]           <- collapses the mean over queries
    emb  = (w^T B) @ Wv + bv            <- collapses attn@V and the V projection

Distribution: 64 pairs -> 8 slots x 8 cores (one shared SPMD program, shapes
fixed per slot to the max over cores; pairs bin-packed by length so padding is
small).

Perf notes vs the first version:
  - A/B uploaded BOTH pre-transposed (DIM-major, fp8e4m3, DIM zero-padded to
    768) for the Q/K path AND natural-layout bf16 for the value path; no
    on-device transposes at all.
  - Projections and QK^T run as fp8 DoubleRow matmuls (2 contraction rows per
    partition, 0.5 cyc/row).  INNER=256 = 2x128 maps exactly onto the
    DoubleRow pair dim for the scores.
  - Value path (w^T B, Wv^T u) in bf16 (1 cyc/row, no small-N penalty).
  - exp() is one activation per q-tile over a [128, plk] PSUM span with a
    single accumulator read for den.
"""

import os
import sys

sys.path.insert(0, "/opt/trn_rl_repo")

import numpy as np

B, LA, LB, DIM, INNER, OUTER = 64, 1024, 1024, 640, 256, 1024
NCORES, NSLOTS, P = 8, 8, 128
SCALE = 1.0 / np.sqrt(INNER)
DT = DIM // P       # 5 d-chunks of 128
DJ = 3              # DoubleRow d-pair chunks (768 = 3 * 256)
DPAD = DJ * 2 * P   # 768

LAST_EXEC_TIME_NS = None


def _chunks(total, step=512):
    out, off = [], 0
    while off < total:
        c = min(step, total - off)
        out.append((off, c))
        off += c
    return out


def _plan(la_all, lb_all):
    """Assign pairs to (slot, core); returns swap flags, groups, slot tile shapes."""
    la = np.asarray(la_all, np.int64)
    lb = np.asarray(lb_all, np.int64)
    swap = lb > la
    qa = np.where(swap, lb, la)  # kernel A-side length (>= B-side)
    qb = np.where(swap, la, lb)
    at = -(-qa // P)
    bt = -(-qb // P)
    order = np.argsort(-(at * 1024 + bt), kind="stable")
    groups = [list(order[s * NCORES:(s + 1) * NCORES]) for s in range(NSLOTS)]
    C1, C2 = 2000.0, 200.0

    def gcost(g):
        ma = max(at[i] for i in g)
        mb = max(bt[i] for i in g)
        return C1 * (ma + mb) + C2 * ma * mb

    rng = np.random.default_rng(0)
    cost = [gcost(g) for g in groups]
    s1s = rng.integers(0, NSLOTS, 30000)
    s2s = rng.integers(0, NSLOTS, 30000)
    i1s = rng.integers(0, NCORES, 30000)
    i2s = rng.integers(0, NCORES, 30000)
    for s1, s2, i1, i2 in zip(s1s, s2s, i1s, i2s):
        if s1 == s2:
            continue
        g1 = groups[s1][:]
        g2 = groups[s2][:]
        g1[i1], g2[i2] = groups[s2][i2], groups[s1][i1]
        n1, n2 = gcost(g1), gcost(g2)
        if n1 + n2 < cost[s1] + cost[s2] - 1e-9:
            groups[s1], groups[s2] = g1, g2
            cost[s1], cost[s2] = n1, n2
    slot_at = [int(max(at[i] for i in g)) for g in groups]
    slot_bt = [int(max(bt[i] for i in g)) for g in groups]
    return swap, qa, qb, groups, slot_at, slot_bt


def _build_program(slot_at, slot_bt):
    import concourse.bass as bass  # noqa: F401
    import concourse.mybir as mybir
    import concourse.tile as tile
    from concourse import bacc

    F32 = mybir.dt.float32
    F32R = mybir.dt.float32r
    BF16 = mybir.dt.bfloat16
    FP8 = mybir.dt.float8e4
    Exp = mybir.ActivationFunctionType.Exp
    Ident = mybir.ActivationFunctionType.Identity
    DR = mybir.MatmulPerfMode.DoubleRow
    Alu = mybir.AluOpType

    tot_at = sum(slot_at)
    tot_bt = sum(slot_bt)
    cum_at = np.concatenate([[0], np.cumsum(slot_at)]).astype(int)
    cum_bt = np.concatenate([[0], np.cumsum(slot_bt)]).astype(int)

    nc = bacc.Bacc("TRN2", target_bir_lowering=False, debug=False,
                   num_devices=NCORES)

    # transposed fp8 inputs: [p, j, i, tok] = X[tok, j*256 + i*128 + p]
    at8_d = nc.dram_tensor("at8", [P, DJ, 2, tot_at * P], FP8,
                           kind="ExternalInput")
    bt8_d = nc.dram_tensor("bt8", [P, DJ, 2, tot_bt * P], FP8,
                           kind="ExternalInput")
    # natural bf16 inputs: [p, T, d] = X[T*128 + p, d]
    an_d = nc.dram_tensor("an16", [P, tot_at, DIM], BF16, kind="ExternalInput")
    bn_d = nc.dram_tensor("bn16", [P, tot_bt, DIM], BF16, kind="ExternalInput")
    gs_a_d = nc.dram_tensor("gs_a", [P, tot_at], F32, kind="ExternalInput")
    gs_b_d = nc.dram_tensor("gs_b", [P, tot_bt], F32, kind="ExternalInput")
    npa_d = nc.dram_tensor("npa", [P, NSLOTS], F32, kind="ExternalInput")
    npb_d = nc.dram_tensor("npb", [P, NSLOTS], F32, kind="ExternalInput")
    wq_d = nc.dram_tensor("wq8", [P, DJ, 2, INNER], FP8, kind="ExternalInput")
    wk_d = nc.dram_tensor("wk8", [P, DJ, 2, INNER], FP8, kind="ExternalInput")
    wv_d = nc.dram_tensor("wv16", [P, DT, OUTER], BF16, kind="ExternalInput")
    bqs_d = nc.dram_tensor("bqs", [P, INNER // P], F32, kind="ExternalInput")
    bv_d = nc.dram_tensor("bv", [P, OUTER // P], F32, kind="ExternalInput")
    idr_d = nc.dram_tensor("idr", [P, P], F32R, kind="ExternalInput")
    idb_d = nc.dram_tensor("idb", [P, P], BF16, kind="ExternalInput")
    emb_d = nc.dram_tensor("emb", [P, OUTER // P, 2 * NSLOTS], F32,
                           kind="ExternalOutput")

    with tile.TileContext(nc) as tc:
        with (
            tc.tile_pool(name="const", bufs=1) as cpool,
            tc.tile_pool(name="ain", bufs=3) as apool,
            tc.tile_pool(name="proj", bufs=2) as ppool,
            tc.tile_pool(name="pexp", bufs=8) as epool,
            tc.tile_pool(name="small", bufs=5) as spool,
            tc.tile_pool(name="late", bufs=2) as lpool,
            tc.tile_pool(name="psA", bufs=3, space="PSUM") as psA,
            tc.tile_pool(name="psW", bufs=1, space="PSUM") as psW,
        ):
            # ---- constants ----
            wq_sb = cpool.tile([P, DJ, 2, INNER], FP8, tag="wq")
            wk_sb = cpool.tile([P, DJ, 2, INNER], FP8, tag="wk")
            bqs_sb = cpool.tile([P, INNER // P], F32, tag="bqs")
            bv_sb = cpool.tile([P, OUTER // P], F32, tag="bv")
            idr_sb = cpool.tile([P, P], F32R, tag="idr")
            npa_sb = cpool.tile([P, NSLOTS], F32, tag="npa")
            npb_sb = cpool.tile([P, NSLOTS], F32, tag="npb")
            gs_a_sb = cpool.tile([P, tot_at], F32, tag="gsa")
            gs_b_sb = cpool.tile([P, tot_bt], F32, tag="gsb")
            wv_sb = cpool.tile([P, DT, OUTER], BF16, tag="wv")
            urows_sb = cpool.tile([2 * NSLOTS, DIM], F32R, tag="urows")
            for sb, d in ((wq_sb, wq_d), (wk_sb, wk_d), (bqs_sb, bqs_d),
                          (bv_sb, bv_d), (idr_sb, idr_d),
                          (npa_sb, npa_d), (npb_sb, npb_d),
                          (gs_a_sb, gs_a_d), (gs_b_sb, gs_b_d),
                          (wv_sb, wv_d)):
                nc.sync.dma_start(sb[:], d[:])

            inbufs = {}

            def load_slot(s):
                at_s, bt_s = int(slot_at[s]), int(slot_bt[s])
                a8 = apool.tile([P, DJ, 2, at_s * P], FP8, tag="a8")
                b8 = apool.tile([P, DJ, 2, bt_s * P], FP8, tag="b8")
                an = apool.tile([P, at_s, DIM], BF16, tag="an")
                bn = apool.tile([P, bt_s, DIM], BF16, tag="bn")
                nc.sync.dma_start(
                    a8[:], at8_d[:, :, :, cum_at[s] * P:(cum_at[s] + at_s) * P])
                nc.sync.dma_start(
                    b8[:], bt8_d[:, :, :, cum_bt[s] * P:(cum_bt[s] + bt_s) * P])
                nc.sync.dma_start(an[:], an_d[:, cum_at[s]:cum_at[s] + at_s, :])
                nc.sync.dma_start(bn[:], bn_d[:, cum_bt[s]:cum_bt[s] + bt_s, :])
                inbufs[s] = (a8, b8, an, bn)

            projbufs = {}

            def proj_gen(s):
                """fp8 DoubleRow projections: qT/kT [p, m, tok]."""
                at_s, bt_s = int(slot_at[s]), int(slot_bt[s])
                pla, plb = at_s * P, bt_s * P
                a8, b8, an, bn = inbufs.pop(s)
                qaT = ppool.tile([P, 2, pla], FP8, tag="qaT")
                kaT = ppool.tile([P, 2, pla], FP8, tag="kaT")
                qbT = ppool.tile([P, 2, plb], FP8, tag="qbT")
                kbT = ppool.tile([P, 2, plb], FP8, tag="kbT")
                projbufs[s] = (qaT, kaT, qbT, kbT, an, bn)
                for src, pl, dst, w_sb, is_q in (
                        (a8, pla, qaT, wq_sb, True),
                        (a8, pla, kaT, wk_sb, False),
                        (b8, plb, qbT, wq_sb, True),
                        (b8, plb, kbT, wk_sb, False)):
                    for m in range(2):
                        pp = psA.tile([P, 1024], F32, tag="mm")
                        for j in range(DJ):
                            for co, cl in _chunks(pl):
                                nc.tensor.matmul(
                                    pp[:, co:co + cl],
                                    w_sb[:, j, :, m * P:(m + 1) * P],
                                    src[:, j, :, co:co + cl],
                                    start=(j == 0), stop=(j == DJ - 1),
                                    perf_mode=DR)
                        if is_q:
                            # q = psum * scale + bq*scale   (fp8 out)
                            nc.vector.tensor_scalar(
                                dst[:, m, :], pp[:, :pl],
                                SCALE, bqs_sb[:, m, None],
                                Alu.mult, Alu.add)
                        else:
                            # k = psum (no bias; softmax shift-invariance)
                            nc.vector.tensor_copy(dst[:, m, :], pp[:, :pl])
                        yield

            def attn_gen(s):
                at_s, bt_s = int(slot_at[s]), int(slot_bt[s])
                qaT, kaT, qbT, kbT, an, bn = projbufs.pop(s)
                for dr in range(2):
                    if dr == 0:  # A queries over B keys
                        QT, KT, nq, nk = qaT, kbT, at_s, bt_s
                        g_sb, g_off = gs_a_sb, cum_at[s]
                        np_sb = npb_sb
                        knat = bn
                    else:
                        QT, KT, nq, nk = qbT, kaT, bt_s, at_s
                        g_sb, g_off = gs_b_sb, cum_bt[s]
                        np_sb = npa_sb
                        knat = an
                    plk = nk * P
                    kch = _chunks(plk)
                    wr = psW.tile([1, 1024], F32, tag="wr")
                    den2 = None
                    pending = []  # [(q0, qt, gcol2, {qp: p_tile})]

                    def issue_wacc(item):
                        q0, qn, gcol2, ptiles = item
                        for qp in range(q0, qn + 1):
                            pt = ptiles[qp]
                            for co, cl in kch:
                                nc.tensor.matmul(
                                    wr[0:1, co:co + cl],
                                    gcol2[:, qp - q0:qp - q0 + 1],
                                    pt[:, co:co + cl],
                                    start=(qp == 0), stop=(qp == nq - 1))

                    p_tiles = {}
                    for qt in range(nq):
                        sc = psA.tile([P, 1024], F32, tag="mm")
                        for co, cl in kch:
                            nc.tensor.matmul(
                                sc[:, co:co + cl],
                                QT[:, :, qt * P:(qt + 1) * P],
                                KT[:, :, co:co + cl],
                                start=True, stop=True, perf_mode=DR)
                        if qt % 2 == 0:
                            den2 = spool.tile([P, 2], F32, tag="den")
                        p_sb = epool.tile([P, 1024], F32R, tag="p")
                        p_tiles[qt] = p_sb
                        nc.scalar.activation(
                            p_sb[:, :plk], sc[:, :plk], Exp,
                            accum_out=den2[:, qt % 2:qt % 2 + 1])
                        if qt % 2 == 1 or qt == nq - 1:
                            q0 = qt - (qt % 2)
                            npair = qt - q0 + 1
                            dpair = den2[:, :npair]
                            # den -= pad count (pad cols are exactly exp(0)=1)
                            nc.vector.tensor_scalar_sub(
                                dpair, dpair, np_sb[:, s:s + 1])
                            rec2 = spool.tile([P, 2], F32, tag="rec")
                            nc.vector.reciprocal(rec2[:, :npair], dpair)
                            gcol2 = spool.tile([P, 2], F32R, tag="gc")
                            nc.vector.tensor_tensor(
                                gcol2[:, :npair], rec2[:, :npair],
                                g_sb[:, g_off + q0:g_off + q0 + npair],
                                Alu.mult)
                            pending.append((q0, qt, gcol2, p_tiles))
                            p_tiles = {}
                            # delay wacc ~2 pairs so the scalar/vector chain
                            # never stalls the in-order PE queue
                            if len(pending) > 2:
                                issue_wacc(pending.pop(0))
                        yield
                    while pending:
                        issue_wacc(pending.pop(0))
                    # w row -> w col (transpose via identity matmul)
                    wrow = lpool.tile([1, 1024], F32R, tag="wrow")
                    if dr == 0:
                        nc.scalar.copy(wrow[0:1, :plk], wr[0:1, :plk])
                    else:
                        nc.vector.tensor_copy(wrow[0:1, :plk], wr[0:1, :plk])
                    wt = psA.tile([P, 1024], F32, tag="mm")
                    for kt in range(nk):
                        nc.tensor.matmul(
                            wt[:, 2 * kt:2 * kt + 2],
                            wrow[0:1, kt * P:(kt + 1) * P],
                            idr_sb[0:1, 0:2], start=True, stop=True)
                    wcol = spool.tile([P, 8], BF16, tag="wcol")
                    nc.vector.tensor_copy(
                        wcol[:, :nk],
                        wt[:, :2 * nk].rearrange(
                            "p (k two) -> p k two", two=2)[:, :, 0])
                    # u row = w^T @ Knat   (bf16)
                    ur = psW.tile([1, 1024], F32, tag="wr")
                    for co, cl in _chunks(DIM):
                        for kt in range(nk):
                            nc.tensor.matmul(
                                ur[0:1, co:co + cl],
                                wcol[:, kt:kt + 1],
                                knat[:, kt, co:co + cl],
                                start=(kt == 0), stop=(kt == nk - 1))
                    ursb = lpool.tile([1, DIM], F32R, tag="ursb")
                    if dr == 0:
                        nc.vector.tensor_copy(ursb[:], ur[0:1, :DIM])
                    else:
                        nc.scalar.copy(ursb[:], ur[0:1, :DIM])
                    nc.sync.dma_start(urows_sb[2 * s + dr:2 * s + dr + 1, :],
                                      ursb[:])
                    yield

            # software pipeline: slot s attention interleaved with slot s+1
            # projections; input DMA prefetched two slots ahead
            load_slot(0)
            load_slot(1)
            for _ in proj_gen(0):
                pass
            for s in range(NSLOTS):
                if s + 2 < NSLOTS:
                    load_slot(s + 2)
                ag = attn_gen(s)
                pg = proj_gen(s + 1) if s + 1 < NSLOTS else None
                for _ in ag:
                    if pg is not None:
                        if next(pg, StopIteration) is StopIteration:
                            pg = None
                if pg is not None:
                    for _ in pg:
                        pass

            # ---- final: E = Wv^T U + bv ----
            u_sb = cpool.tile([P, DT, 2 * NSLOTS], BF16, tag="usb")
            for dt in range(DT):
                ut = psA.tile([P, 1024], F32, tag="mm")
                nc.tensor.matmul(
                    ut[:, :2 * NSLOTS],
                    urows_sb[:, dt * P:(dt + 1) * P],
                    idr_sb[0:2 * NSLOTS, 0:2 * NSLOTS],
                    start=True, stop=True)
                nc.vector.tensor_copy(u_sb[:, dt, :], ut[:, :2 * NSLOTS])
            e_sb = cpool.tile([P, OUTER // P, 2 * NSLOTS], F32, tag="esb")
            for oc in range(OUTER // P):
                ep = psA.tile([P, 1024], F32, tag="mm")
                for dt in range(DT):
                    nc.tensor.matmul(
                        ep[:, :2 * NSLOTS],
                        wv_sb[:, dt, oc * P:(oc + 1) * P],
                        u_sb[:, dt, :],
                        start=(dt == 0), stop=(dt == DT - 1))
                nc.vector.tensor_scalar_add(e_sb[:, oc, :], ep[:, :2 * NSLOTS],
                                            bv_sb[:, oc, None])
            nc.sync.dma_start(emb_d[:], e_sb[:])

    nc.compile()
    return nc


def _install_profhook():
    import contextlib
    import ctypes
    import types

    import antenv

    if not hasattr(antenv, "axon_hooks"):
        mod = types.ModuleType("antenv.axon_hooks")
        mod._hook = None

        def _set(h):
            mod._hook = h

        def _get():
            return mod._hook

        mod.set_axon_ntff_profile_hook = _set
        mod.get_axon_ntff_profile_hook = _get
        sys.modules["antenv.axon_hooks"] = mod
        antenv.axon_hooks = mod
    from antenv.axon_hooks import set_axon_ntff_profile_hook
    so_path = "/opt/axon/libaxon_pjrt.so"
    if not os.path.exists(so_path):
        return False
    lib = ctypes.CDLL(so_path)
    if not hasattr(lib, "axon_start_nrt_profile"):
        return False
    lib.axon_start_nrt_profile.argtypes = [ctypes.POINTER(ctypes.c_int64),
                                           ctypes.c_size_t]
    lib.axon_start_nrt_profile.restype = ctypes.c_int64
    lib.axon_stop_nrt_profile.argtypes = [ctypes.c_char_p]
    lib.axon_stop_nrt_profile.restype = ctypes.c_int64

    @contextlib.contextmanager
    def _hook(output_dir, device_ids):
        import jax

        jax.devices()
        if device_ids:
            ids = (ctypes.c_int64 * len(device_ids))(*device_ids)
            rc = lib.axon_start_nrt_profile(ids, len(device_ids))
        else:
            rc = lib.axon_start_nrt_profile(None, 0)
        if rc != 0:
            raise RuntimeError(f"axon_start_nrt_profile rc={rc}")
        try:
            yield
        finally:
            n = lib.axon_stop_nrt_profile(str(output_dir).encode())
            print(f"profile: {n} file(s) written to {output_dir}",
                  file=sys.stderr)

    set_axon_ntff_profile_hook(_hook)
    return True


def kernel(a_pad, b_pad, len_a, len_b, Wq, bq, Wk, bk, Wv, bv):
    global LAST_EXEC_TIME_NS
    import ml_dtypes
    FP8 = ml_dtypes.float8_e4m3fn
    BF16 = ml_dtypes.bfloat16

    a_pad = np.ascontiguousarray(np.asarray(a_pad, np.float32))
    b_pad = np.ascontiguousarray(np.asarray(b_pad, np.float32))
    len_a = np.asarray(len_a, np.int32)
    len_b = np.asarray(len_b, np.int32)
    Wq = np.asarray(Wq, np.float32)
    Wk = np.asarray(Wk, np.float32)
    Wv = np.asarray(Wv, np.float32)
    bq = np.asarray(bq, np.float32)
    bv = np.asarray(bv, np.float32)

    swap, qa_len, qb_len, groups, slot_at, slot_bt = _plan(len_a, len_b)
    tot_at, tot_bt = sum(slot_at), sum(slot_bt)
    cum_at = np.concatenate([[0], np.cumsum(slot_at)]).astype(int)
    cum_bt = np.concatenate([[0], np.cumsum(slot_bt)]).astype(int)

    # ---- shared (per-core-identical) inputs ----
    def pack_w8(W):
        # [640, INNER] -> [128, 3, 2, INNER] with d = j*256 + i*128 + p
        Wp = np.zeros((DPAD, W.shape[1]), np.float32)
        Wp[:DIM] = W
        return np.ascontiguousarray(
            Wp.reshape(DJ, 2, P, W.shape[1]).transpose(2, 0, 1, 3)
        ).astype(FP8)

    wq8 = pack_w8(Wq)
    wk8 = pack_w8(Wk)
    wv16 = np.ascontiguousarray(
        Wv.reshape(DT, P, OUTER).transpose(1, 0, 2)).astype(BF16)
    bqs_h = (bq * SCALE).reshape(INNER // P, P).T.copy()
    bv_h = bv.reshape(OUTER // P, P).T.copy()
    idr_h = np.eye(P, dtype=np.float32)
    idb_h = np.eye(P, dtype=np.float32).astype(BF16)

    # ---- per-core inputs ----
    in_maps = []
    for c in range(NCORES):
        abuf = np.zeros((tot_at * P, DPAD), np.float32)
        bbuf = np.zeros((tot_bt * P, DPAD), np.float32)
        gs_a = np.zeros((P, tot_at), np.float32)
        gs_b = np.zeros((P, tot_bt), np.float32)
        npa = np.zeros((P, NSLOTS), np.float32)
        npb = np.zeros((P, NSLOTS), np.float32)
        for s in range(NSLOTS):
            i = groups[s][c]
            la_i, lb_i = int(qa_len[i]), int(qb_len[i])
            A = b_pad[i] if swap[i] else a_pad[i]
            Bm = a_pad[i] if swap[i] else b_pad[i]
            abuf[cum_at[s] * P:cum_at[s] * P + la_i, :DIM] = A[:la_i]
            bbuf[cum_bt[s] * P:cum_bt[s] * P + lb_i, :DIM] = Bm[:lb_i]
            ga = np.zeros(slot_at[s] * P, np.float32)
            ga[:la_i] = 1.0 / la_i
            gs_a[:, cum_at[s]:cum_at[s] + slot_at[s]] = \
                ga.reshape(slot_at[s], P).T
            gb = np.zeros(slot_bt[s] * P, np.float32)
            gb[:lb_i] = 1.0 / lb_i
            gs_b[:, cum_bt[s]:cum_bt[s] + slot_bt[s]] = \
                gb.reshape(slot_bt[s], P).T
            npa[:, s] = slot_at[s] * P - la_i
            npb[:, s] = slot_bt[s] * P - lb_i
        # transposed fp8: [tok, 768] -> [128, 3, 2, tok]
        at8 = np.ascontiguousarray(
            abuf.reshape(tot_at * P, DJ, 2, P).transpose(3, 1, 2, 0)
        ).astype(FP8)
        bt8 = np.ascontiguousarray(
            bbuf.reshape(tot_bt * P, DJ, 2, P).transpose(3, 1, 2, 0)
        ).astype(FP8)
        # natural bf16: [tok, 640] -> [128, T, 640]
        an16 = np.ascontiguousarray(
            abuf[:, :DIM].reshape(tot_at, P, DIM).transpose(1, 0, 2)
        ).astype(BF16)
        bn16 = np.ascontiguousarray(
            bbuf[:, :DIM].reshape(tot_bt, P, DIM).transpose(1, 0, 2)
        ).astype(BF16)
        in_maps.append({
            "at8": at8, "bt8": bt8, "an16": an16, "bn16": bn16,
            "gs_a": gs_a, "gs_b": gs_b, "npa": npa, "npb": npb,
            "wq8": wq8, "wk8": wk8, "wv16": wv16,
            "bqs": bqs_h, "bv": bv_h, "idr": idr_h, "idb": idb_h,
        })

    nc = _build_program(slot_at, slot_bt)

    from concourse.bass_utils import run_bass_kernel_spmd

    trace = os.environ.get("BASS_KERNEL_TRACE", "0") == "1"
    if trace:
        _install_profhook()
    res = run_bass_kernel_spmd(nc, in_maps, list(range(NCORES)), trace=trace)
    LAST_EXEC_TIME_NS = res.exec_time_ns

    emb_a = np.zeros((B, OUTER), np.float32)
    emb_b = np.zeros((B, OUTER), np.float32)
    for c in range(NCORES):
        e = res.results[c]["emb"].transpose(1, 0, 2).reshape(OUTER,
                                                            2 * NSLOTS)
        for s in range(NSLOTS):
            i = groups[s][c]
            ea, eb = e[:, 2 * s], e[:, 2 * s + 1]  # A-queries, B-queries
            if swap[i]:
                emb_a[i], emb_b[i] = eb, ea
            else:
                emb_a[i], emb_b[i] = ea, eb
    return emb_a, emb_b


# revision 12
# speedup vs baseline: 1.8419x; 1.0426x over previous
"""Ragged cross-attention pooling kernel for Trainium2 (8 NeuronCores, SPMD).

Math (per pair, direction "A attends over B"):
    qa = (A @ Wq + bq) * scale          [la, INNER]
    kb =  B @ Wk                        [lb, INNER]   (bk dropped: softmax
                                                       is shift-invariant per query)
    s  = qa @ kb^T                      [la, lb]      (pad k-cols are exactly 0)
    p  = exp(s)                                       (pad cols: exp(0) = 1.0)
    den[q] = sum_k p[q, k] - n_pad                    (exact pad correction)
    g[q] = valid(q) / (la * den[q])
    w[k] = sum_q g[q] p[q, k]           <- collapses the mean over queries
    emb  = (w^T B) @ Wv + bv            <- collapses attn@V and the V projection

Distribution: 64 pairs -> 8 slots x 8 cores (one shared SPMD program, shapes
fixed per slot to the max over cores; pairs bin-packed by length so padding is
small).

Perf notes vs the first version:
  - A/B uploaded BOTH pre-transposed (DIM-major, fp8e4m3, DIM zero-padded to
    768) for the Q/K path AND natural-layout bf16 for the value path; no
    on-device transposes at all.
  - Projections and QK^T run as fp8 DoubleRow matmuls (2 contraction rows per
    partition, 0.5 cyc/row).  INNER=256 = 2x128 maps exactly onto the
    DoubleRow pair dim for the scores.
  - Value path (w^T B, Wv^T u) in bf16 (1 cyc/row, no small-N penalty).
  - exp() is one activation per q-tile over a [128, plk] PSUM span with a
    single accumulator read for den.
"""

import os
import sys

sys.path.insert(0, "/opt/trn_rl_repo")

import numpy as np

B, LA, LB, DIM, INNER, OUTER = 64, 1024, 1024, 640, 256, 1024
NCORES, NSLOTS, P = 8, 8, 128
SCALE = 1.0 / np.sqrt(INNER)
DT = DIM // P       # 5 d-chunks of 128
DJ = 3              # DoubleRow d-pair chunks (768 = 3 * 256)
DPAD = DJ * 2 * P   # 768

LAST_EXEC_TIME_NS = None


def _chunks(total, step=512):
    out, off = [], 0
    while off < total:
        c = min(step, total - off)
        out.append((off, c))
        off += c
    return out


def _plan(la_all, lb_all):
    """Assign pairs to (slot, core); returns swap flags, groups, slot tile shapes."""
    la = np.asarray(la_all, np.int64)
    lb = np.asarray(lb_all, np.int64)
    swap = lb > la
    qa = np.where(swap, lb, la)  # kernel A-side length (>= B-side)
    qb = np.where(swap, la, lb)
    at = -(-qa // P)
    bt = -(-qb // P)
    order = np.argsort(-(at * 1024 + bt), kind="stable")
    groups = [list(order[s * NCORES:(s + 1) * NCORES]) for s in range(NSLOTS)]
    C1, C2 = 2000.0, 200.0

    def gcost(g):
        ma = max(at[i] for i in g)
        mb = max(bt[i] for i in g)
        return C1 * (ma + mb) + C2 * ma * mb

    rng = np.random.default_rng(0)
    cost = [gcost(g) for g in groups]
    s1s = rng.integers(0, NSLOTS, 30000)
    s2s = rng.integers(0, NSLOTS, 30000)
    i1s = rng.integers(0, NCORES, 30000)
    i2s = rng.integers(0, NCORES, 30000)
    for s1, s2, i1, i2 in zip(s1s, s2s, i1s, i2s):
        if s1 == s2:
            continue
        g1 = groups[s1][:]
        g2 = groups[s2][:]
        g1[i1], g2[i2] = groups[s2][i2], groups[s1][i1]
        n1, n2 = gcost(g1), gcost(g2)
        if n1 + n2 < cost[s1] + cost[s2] - 1e-9:
            groups[s1], groups[s2] = g1, g2
            cost[s1], cost[s2] = n1, n2
    slot_at = [int(max(at[i] for i in g)) for g in groups]
    slot_bt = [int(max(bt[i] for i in g)) for g in groups]
    return swap, qa, qb, groups, slot_at, slot_bt


def _build_program(slot_at, slot_bt):
    import concourse.bass as bass  # noqa: F401
    import concourse.mybir as mybir
    import concourse.tile as tile
    from concourse import bacc

    F32 = mybir.dt.float32
    F32R = mybir.dt.float32r
    BF16 = mybir.dt.bfloat16
    FP8 = mybir.dt.float8e4
    Exp = mybir.ActivationFunctionType.Exp
    Ident = mybir.ActivationFunctionType.Identity
    DR = mybir.MatmulPerfMode.DoubleRow
    Alu = mybir.AluOpType

    tot_at = sum(slot_at)
    tot_bt = sum(slot_bt)
    cum_at = np.concatenate([[0], np.cumsum(slot_at)]).astype(int)
    cum_bt = np.concatenate([[0], np.cumsum(slot_bt)]).astype(int)

    nc = bacc.Bacc("TRN2", target_bir_lowering=False, debug=False,
                   num_devices=NCORES)

    # transposed fp8 inputs: [p, j, i, tok] = X[tok, j*256 + i*128 + p]
    at8_d = nc.dram_tensor("at8", [P, DJ, 2, tot_at * P], FP8,
                           kind="ExternalInput")
    bt8_d = nc.dram_tensor("bt8", [P, DJ, 2, tot_bt * P], FP8,
                           kind="ExternalInput")
    # natural bf16 inputs: [p, T, d] = X[T*128 + p, d]
    an_d = nc.dram_tensor("an16", [P, tot_at, DIM], BF16, kind="ExternalInput")
    bn_d = nc.dram_tensor("bn16", [P, tot_bt, DIM], BF16, kind="ExternalInput")
    gs_a_d = nc.dram_tensor("gs_a", [P, tot_at], F32, kind="ExternalInput")
    gs_b_d = nc.dram_tensor("gs_b", [P, tot_bt], F32, kind="ExternalInput")
    npa_d = nc.dram_tensor("npa", [P, NSLOTS], F32, kind="ExternalInput")
    npb_d = nc.dram_tensor("npb", [P, NSLOTS], F32, kind="ExternalInput")
    wq_d = nc.dram_tensor("wq8", [P, DJ, 2, INNER], FP8, kind="ExternalInput")
    wk_d = nc.dram_tensor("wk8", [P, DJ, 2, INNER], FP8, kind="ExternalInput")
    wv_d = nc.dram_tensor("wv16", [P, DT, OUTER], BF16, kind="ExternalInput")
    bqs_d = nc.dram_tensor("bqs", [P, INNER // P], F32, kind="ExternalInput")
    bv_d = nc.dram_tensor("bv", [P, OUTER // P], F32, kind="ExternalInput")
    idr_d = nc.dram_tensor("idr", [P, P], F32R, kind="ExternalInput")
    idb_d = nc.dram_tensor("idb", [P, P], BF16, kind="ExternalInput")
    emb_d = nc.dram_tensor("emb", [P, OUTER // P, 2 * NSLOTS], F32,
                           kind="ExternalOutput")

    with tile.TileContext(nc) as tc:
        with (
            tc.tile_pool(name="const", bufs=1) as cpool,
            tc.tile_pool(name="ain", bufs=3) as apool,
            tc.tile_pool(name="proj", bufs=2) as ppool,
            tc.tile_pool(name="pexp", bufs=8) as epool,
            tc.tile_pool(name="small", bufs=5) as spool,
            tc.tile_pool(name="late", bufs=3) as lpool,
            tc.tile_pool(name="psA", bufs=3, space="PSUM") as psA,
            tc.tile_pool(name="psW", bufs=1, space="PSUM") as psW,
        ):
            # ---- constants (DMA-ordered: slot-0 critical path first) ----
            wq_sb = cpool.tile([P, DJ, 2, INNER], FP8, tag="wq")
            wk_sb = cpool.tile([P, DJ, 2, INNER], FP8, tag="wk")
            bqs_sb = cpool.tile([P, INNER // P], F32, tag="bqs")
            bv_sb = cpool.tile([P, OUTER // P], F32, tag="bv")
            idr_sb = cpool.tile([P, P], F32R, tag="idr")
            npa_sb = cpool.tile([P, NSLOTS], F32, tag="npa")
            npb_sb = cpool.tile([P, NSLOTS], F32, tag="npb")
            gs_a_sb = cpool.tile([P, tot_at], F32, tag="gsa")
            gs_b_sb = cpool.tile([P, tot_bt], F32, tag="gsb")
            wv_sb = cpool.tile([P, DT, OUTER], BF16, tag="wv")
            urows_sb = cpool.tile([2 * NSLOTS, DIM], F32R, tag="urows")
            for sb, d in ((wq_sb, wq_d), (wk_sb, wk_d), (bqs_sb, bqs_d)):
                nc.sync.dma_start(sb[:], d[:])

            inbufs = {}

            def load_slot(s, qk_only=False, nat_only=False):
                at_s, bt_s = int(slot_at[s]), int(slot_bt[s])
                if not nat_only:
                    a8 = apool.tile([P, DJ, 2, at_s * P], FP8, tag="a8")
                    b8 = apool.tile([P, DJ, 2, bt_s * P], FP8, tag="b8")
                    nc.sync.dma_start(
                        a8[:],
                        at8_d[:, :, :, cum_at[s] * P:(cum_at[s] + at_s) * P])
                    nc.sync.dma_start(
                        b8[:],
                        bt8_d[:, :, :, cum_bt[s] * P:(cum_bt[s] + bt_s) * P])
                    inbufs[s] = (a8, b8, None, None)
                if not qk_only:
                    an = apool.tile([P, at_s, DIM], BF16, tag="an")
                    bn = apool.tile([P, bt_s, DIM], BF16, tag="bn")
                    nc.sync.dma_start(an[:],
                                      an_d[:, cum_at[s]:cum_at[s] + at_s, :])
                    nc.sync.dma_start(bn[:],
                                      bn_d[:, cum_bt[s]:cum_bt[s] + bt_s, :])
                    a8, b8, _, _ = inbufs[s]
                    inbufs[s] = (a8, b8, an, bn)

            projbufs = {}

            def proj_gen(s):
                """fp8 DoubleRow projections: qT/kT [p, m, tok]."""
                at_s, bt_s = int(slot_at[s]), int(slot_bt[s])
                pla, plb = at_s * P, bt_s * P
                a8, b8, an, bn = inbufs.pop(s)
                qaT = ppool.tile([P, 2, pla], FP8, tag="qaT")
                kaT = ppool.tile([P, 2, pla], FP8, tag="kaT")
                qbT = ppool.tile([P, 2, plb], FP8, tag="qbT")
                kbT = ppool.tile([P, 2, plb], FP8, tag="kbT")
                projbufs[s] = (qaT, kaT, qbT, kbT, an, bn)
                for src, pl, dst, w_sb, is_q in (
                        (a8, pla, qaT, wq_sb, True),
                        (a8, pla, kaT, wk_sb, False),
                        (b8, plb, qbT, wq_sb, True),
                        (b8, plb, kbT, wk_sb, False)):
                    for m in range(2):
                        pp = psA.tile([P, 1024], F32, tag="mm")
                        for j in range(DJ):
                            for co, cl in _chunks(pl):
                                nc.tensor.matmul(
                                    pp[:, co:co + cl],
                                    w_sb[:, j, :, m * P:(m + 1) * P],
                                    src[:, j, :, co:co + cl],
                                    start=(j == 0), stop=(j == DJ - 1),
                                    perf_mode=DR)
                        if is_q:
                            # q = psum * scale + bq*scale   (fp8 out)
                            nc.vector.tensor_scalar(
                                dst[:, m, :], pp[:, :pl],
                                SCALE, bqs_sb[:, m, None],
                                Alu.mult, Alu.add)
                        else:
                            # k = psum (no bias; softmax shift-invariance)
                            nc.vector.tensor_copy(dst[:, m, :], pp[:, :pl])
                        yield

            def tail_gen(s, dr, wr, plk, nk, knat):
                """Deferred per-direction epilogue: transpose w, compute u.

                First step (run eagerly at direction end): wrow copy, freeing
                the wr psum slot.  Later steps are drained one per q-tile
                of the following direction so the PE never idles on the
                wrow/wcol dependency chain.
                """
                wrow = lpool.tile([1, 1024], F32R, tag="wrow")
                if dr == 0:
                    nc.scalar.copy(wrow[0:1, :plk], wr[0:1, :plk])
                else:
                    nc.vector.tensor_copy(wrow[0:1, :plk], wr[0:1, :plk])
                yield
                wt = psA.tile([P, 1024], F32, tag="mm")
                for kt in range(nk):
                    nc.tensor.matmul(
                        wt[:, 2 * kt:2 * kt + 2],
                        wrow[0:1, kt * P:(kt + 1) * P],
                        idr_sb[0:1, 0:2], start=True, stop=True)
                wcol = spool.tile([P, 8], BF16, tag="wcol")
                nc.vector.tensor_copy(
                    wcol[:, :nk],
                    wt[:, :2 * nk].rearrange(
                        "p (k two) -> p k two", two=2)[:, :, 0])
                yield
                # u row = w^T @ Knat   (bf16); ur reuses the wr psum slot
                ur = psW.tile([1, 1024], F32, tag="wr")
                for co, cl in _chunks(DIM):
                    for kt in range(nk):
                        nc.tensor.matmul(
                            ur[0:1, co:co + cl],
                            wcol[:, kt:kt + 1],
                            knat[:, kt, co:co + cl],
                            start=(kt == 0), stop=(kt == nk - 1))
                ursb = lpool.tile([1, DIM], F32R, tag="ursb")
                if dr == 0:
                    nc.vector.tensor_copy(ursb[:], ur[0:1, :DIM])
                else:
                    nc.scalar.copy(ursb[:], ur[0:1, :DIM])
                nc.sync.dma_start(urows_sb[2 * s + dr:2 * s + dr + 1, :],
                                  ursb[:])

            tails = []  # deferred tail generators, drained one step per yield

            def drain_tail_step():
                if tails:
                    if next(tails[0], StopIteration) is StopIteration:
                        tails.pop(0)

            def attn_gen(s):
                at_s, bt_s = int(slot_at[s]), int(slot_bt[s])
                qaT, kaT, qbT, kbT, an, bn = projbufs.pop(s)
                for dr in range(2):
                    if dr == 0:  # A queries over B keys
                        QT, KT, nq, nk = qaT, kbT, at_s, bt_s
                        g_sb, g_off = gs_a_sb, cum_at[s]
                        np_sb = npb_sb
                        knat = bn
                    else:
                        QT, KT, nq, nk = qbT, kaT, bt_s, at_s
                        g_sb, g_off = gs_b_sb, cum_bt[s]
                        np_sb = npa_sb
                        knat = an
                    plk = nk * P
                    kch = _chunks(plk)
                    wr = psW.tile([1, 1024], F32, tag="wr")
                    den2 = None
                    pending = []  # [(q0, qt, gcol2, {qp: p_tile})]

                    def issue_wacc(item):
                        q0, qn, gcol2, ptiles = item
                        for qp in range(q0, qn + 1):
                            pt = ptiles[qp]
                            for co, cl in kch:
                                nc.tensor.matmul(
                                    wr[0:1, co:co + cl],
                                    gcol2[:, qp - q0:qp - q0 + 1],
                                    pt[:, co:co + cl],
                                    start=(qp == 0), stop=(qp == nq - 1))

                    p_tiles = {}
                    for qt in range(nq):
                        sc = psA.tile([P, 1024], F32, tag="mm")
                        for co, cl in kch:
                            nc.tensor.matmul(
                                sc[:, co:co + cl],
                                QT[:, :, qt * P:(qt + 1) * P],
                                KT[:, :, co:co + cl],
                                start=True, stop=True, perf_mode=DR)
                        if qt % 2 == 0:
                            den2 = spool.tile([P, 2], F32, tag="den")
                        p_sb = epool.tile([P, 1024], F32R, tag="p")
                        p_tiles[qt] = p_sb
                        nc.scalar.activation(
                            p_sb[:, :plk], sc[:, :plk], Exp,
                            accum_out=den2[:, qt % 2:qt % 2 + 1])
                        if qt % 2 == 1 or qt == nq - 1:
                            q0 = qt - (qt % 2)
                            npair = qt - q0 + 1
                            dpair = den2[:, :npair]
                            # den -= pad count (pad cols are exactly exp(0)=1)
                            nc.vector.tensor_scalar_sub(
                                dpair, dpair, np_sb[:, s:s + 1])
                            rec2 = spool.tile([P, 2], F32, tag="rec")
                            nc.vector.reciprocal(rec2[:, :npair], dpair)
                            gcol2 = spool.tile([P, 2], F32R, tag="gc")
                            nc.vector.tensor_tensor(
                                gcol2[:, :npair], rec2[:, :npair],
                                g_sb[:, g_off + q0:g_off + q0 + npair],
                                Alu.mult)
                            pending.append((q0, qt, gcol2, p_tiles))
                            p_tiles = {}
                            # delay wacc ~2 pairs so the scalar/vector chain
                            # never stalls the in-order PE queue
                            if len(pending) > 2:
                                issue_wacc(pending.pop(0))
                        drain_tail_step()
                        yield
                    while pending:
                        issue_wacc(pending.pop(0))
                    tails.append(tail_gen(s, dr, wr, plk, nk, knat))
                    next(tails[-1])  # eager wrow copy; frees wr for next dir
                    yield

            # software pipeline: slot s attention interleaved with slot s+1
            # projections; input DMA prefetched ~two slots ahead; DMA queue
            # ordered so slot-0 projections can start ASAP
            load_slot(0, qk_only=True)
            for sb, d in ((gs_a_sb, gs_a_d), (gs_b_sb, gs_b_d),
                          (npa_sb, npa_d), (npb_sb, npb_d),
                          (idr_sb, idr_d)):
                nc.sync.dma_start(sb[:], d[:])
            load_slot(0, nat_only=True)
            load_slot(1)
            nc.sync.dma_start(bv_sb[:], bv_d[:])
            nc.sync.dma_start(wv_sb[:], wv_d[:])
            for _ in proj_gen(0):
                pass
            for s in range(NSLOTS):
                if s + 2 < NSLOTS:
                    load_slot(s + 2)
                ag = attn_gen(s)
                pg = proj_gen(s + 1) if s + 1 < NSLOTS else None
                for _ in ag:
                    if pg is not None:
                        if next(pg, StopIteration) is StopIteration:
                            pg = None
                if pg is not None:
                    for _ in pg:
                        pass
            while tails:
                drain_tail_step()

            # ---- final: E = Wv^T U + bv ----
            u_sb = cpool.tile([P, DT, 2 * NSLOTS], BF16, tag="usb")
            for dt in range(DT):
                ut = psA.tile([P, 1024], F32, tag="mm")
                nc.tensor.matmul(
                    ut[:, :2 * NSLOTS],
                    urows_sb[:, dt * P:(dt + 1) * P],
                    idr_sb[0:2 * NSLOTS, 0:2 * NSLOTS],
                    start=True, stop=True)
                nc.vector.tensor_copy(u_sb[:, dt, :], ut[:, :2 * NSLOTS])
            e_sb = cpool.tile([P, OUTER // P, 2 * NSLOTS], F32, tag="esb")
            for oc in range(OUTER // P):
                ep = psA.tile([P, 1024], F32, tag="mm")
                for dt in range(DT):
                    nc.tensor.matmul(
                        ep[:, :2 * NSLOTS],
                        wv_sb[:, dt, oc * P:(oc + 1) * P],
                        u_sb[:, dt, :],
                        start=(dt == 0), stop=(dt == DT - 1))
                nc.vector.tensor_scalar_add(e_sb[:, oc, :], ep[:, :2 * NSLOTS],
                                            bv_sb[:, oc, None])
            nc.sync.dma_start(emb_d[:], e_sb[:])

    nc.compile()
    return nc


def _install_profhook():
    import contextlib
    import ctypes
    import types

    import antenv

    if not hasattr(antenv, "axon_hooks"):
        mod = types.ModuleType("antenv.axon_hooks")
        mod._hook = None

        def _set(h):
            mod._hook = h

        def _get():
            return mod._hook

        mod.set_axon_ntff_profile_hook = _set
        mod.get_axon_ntff_profile_hook = _get
        sys.modules["antenv.axon_hooks"] = mod
        antenv.axon_hooks = mod
    from antenv.axon_hooks import set_axon_ntff_profile_hook
    so_path = "/opt/axon/libaxon_pjrt.so"
    if not os.path.exists(so_path):
        return False
    lib = ctypes.CDLL(so_path)
    if not hasattr(lib, "axon_start_nrt_profile"):
        return False
    lib.axon_start_nrt_profile.argtypes = [ctypes.POINTER(ctypes.c_int64),
                                           ctypes.c_size_t]
    lib.axon_start_nrt_profile.restype = ctypes.c_int64
    lib.axon_stop_nrt_profile.argtypes = [ctypes.c_char_p]
    lib.axon_stop_nrt_profile.restype = ctypes.c_int64

    @contextlib.contextmanager
    def _hook(output_dir, device_ids):
        import jax

        jax.devices()
        if device_ids:
            ids = (ctypes.c_int64 * len(device_ids))(*device_ids)
            rc = lib.axon_start_nrt_profile(ids, len(device_ids))
        else:
            rc = lib.axon_start_nrt_profile(None, 0)
        if rc != 0:
            raise RuntimeError(f"axon_start_nrt_profile rc={rc}")
        try:
            yield
        finally:
            n = lib.axon_stop_nrt_profile(str(output_dir).encode())
            print(f"profile: {n} file(s) written to {output_dir}",
                  file=sys.stderr)

    set_axon_ntff_profile_hook(_hook)
    return True


def kernel(a_pad, b_pad, len_a, len_b, Wq, bq, Wk, bk, Wv, bv):
    global LAST_EXEC_TIME_NS
    import ml_dtypes
    FP8 = ml_dtypes.float8_e4m3fn
    BF16 = ml_dtypes.bfloat16

    a_pad = np.ascontiguousarray(np.asarray(a_pad, np.float32))
    b_pad = np.ascontiguousarray(np.asarray(b_pad, np.float32))
    len_a = np.asarray(len_a, np.int32)
    len_b = np.asarray(len_b, np.int32)
    Wq = np.asarray(Wq, np.float32)
    Wk = np.asarray(Wk, np.float32)
    Wv = np.asarray(Wv, np.float32)
    bq = np.asarray(bq, np.float32)
    bv = np.asarray(bv, np.float32)

    swap, qa_len, qb_len, groups, slot_at, slot_bt = _plan(len_a, len_b)
    tot_at, tot_bt = sum(slot_at), sum(slot_bt)
    cum_at = np.concatenate([[0], np.cumsum(slot_at)]).astype(int)
    cum_bt = np.concatenate([[0], np.cumsum(slot_bt)]).astype(int)

    # ---- shared (per-core-identical) inputs ----
    def pack_w8(W):
        # [640, INNER] -> [128, 3, 2, INNER] with d = j*256 + i*128 + p
        Wp = np.zeros((DPAD, W.shape[1]), np.float32)
        Wp[:DIM] = W
        return np.ascontiguousarray(
            Wp.reshape(DJ, 2, P, W.shape[1]).transpose(2, 0, 1, 3)
        ).astype(FP8)

    wq8 = pack_w8(Wq)
    wk8 = pack_w8(Wk)
    wv16 = np.ascontiguousarray(
        Wv.reshape(DT, P, OUTER).transpose(1, 0, 2)).astype(BF16)
    bqs_h = (bq * SCALE).reshape(INNER // P, P).T.copy()
    bv_h = bv.reshape(OUTER // P, P).T.copy()
    idr_h = np.eye(P, dtype=np.float32)
    idb_h = np.eye(P, dtype=np.float32).astype(BF16)

    # ---- per-core inputs ----
    in_maps = []
    for c in range(NCORES):
        abuf = np.zeros((tot_at * P, DPAD), np.float32)
        bbuf = np.zeros((tot_bt * P, DPAD), np.float32)
        gs_a = np.zeros((P, tot_at), np.float32)
        gs_b = np.zeros((P, tot_bt), np.float32)
        npa = np.zeros((P, NSLOTS), np.float32)
        npb = np.zeros((P, NSLOTS), np.float32)
        for s in range(NSLOTS):
            i = groups[s][c]
            la_i, lb_i = int(qa_len[i]), int(qb_len[i])
            A = b_pad[i] if swap[i] else a_pad[i]
            Bm = a_pad[i] if swap[i] else b_pad[i]
            abuf[cum_at[s] * P:cum_at[s] * P + la_i, :DIM] = A[:la_i]
            bbuf[cum_bt[s] * P:cum_bt[s] * P + lb_i, :DIM] = Bm[:lb_i]
            ga = np.zeros(slot_at[s] * P, np.float32)
            ga[:la_i] = 1.0 / la_i
            gs_a[:, cum_at[s]:cum_at[s] + slot_at[s]] = \
                ga.reshape(slot_at[s], P).T
            gb = np.zeros(slot_bt[s] * P, np.float32)
            gb[:lb_i] = 1.0 / lb_i
            gs_b[:, cum_bt[s]:cum_bt[s] + slot_bt[s]] = \
                gb.reshape(slot_bt[s], P).T
            npa[:, s] = slot_at[s] * P - la_i
            npb[:, s] = slot_bt[s] * P - lb_i
        # transposed fp8: [tok, 768] -> [128, 3, 2, tok]
        at8 = np.ascontiguousarray(
            abuf.reshape(tot_at * P, DJ, 2, P).transpose(3, 1, 2, 0)
        ).astype(FP8)
        bt8 = np.ascontiguousarray(
            bbuf.reshape(tot_bt * P, DJ, 2, P).transpose(3, 1, 2, 0)
        ).astype(FP8)
        # natural bf16: [tok, 640] -> [128, T, 640]
        an16 = np.ascontiguousarray(
            abuf[:, :DIM].reshape(tot_at, P, DIM).transpose(1, 0, 2)
        ).astype(BF16)
        bn16 = np.ascontiguousarray(
            bbuf[:, :DIM].reshape(tot_bt, P, DIM).transpose(1, 0, 2)
        ).astype(BF16)
        in_maps.append({
            "at8": at8, "bt8": bt8, "an16": an16, "bn16": bn16,
            "gs_a": gs_a, "gs_b": gs_b, "npa": npa, "npb": npb,
            "wq8": wq8, "wk8": wk8, "wv16": wv16,
            "bqs": bqs_h, "bv": bv_h, "idr": idr_h, "idb": idb_h,
        })

    nc = _build_program(slot_at, slot_bt)

    from concourse.bass_utils import run_bass_kernel_spmd

    trace = os.environ.get("BASS_KERNEL_TRACE", "0") == "1"
    if trace:
        _install_profhook()
    res = run_bass_kernel_spmd(nc, in_maps, list(range(NCORES)), trace=trace)
    LAST_EXEC_TIME_NS = res.exec_time_ns

    emb_a = np.zeros((B, OUTER), np.float32)
    emb_b = np.zeros((B, OUTER), np.float32)
    for c in range(NCORES):
        e = res.results[c]["emb"].transpose(1, 0, 2).reshape(OUTER,
                                                            2 * NSLOTS)
        for s in range(NSLOTS):
            i = groups[s][c]
            ea, eb = e[:, 2 * s], e[:, 2 * s + 1]  # A-queries, B-queries
            if swap[i]:
                emb_a[i], emb_b[i] = eb, ea
            else:
                emb_a[i], emb_b[i] = ea, eb
    return emb_a, emb_b


# revision 17
# speedup vs baseline: 2.0173x; 1.0952x over previous
"""Ragged cross-attention pooling kernel for Trainium2 (8 NeuronCores, SPMD).

Math (per pair, direction "A attends over B"):
    qa = (A @ Wq + bq) * scale          [la, INNER]
    kb =  B @ Wk                        [lb, INNER]   (bk dropped: softmax
                                                       is shift-invariant per query)
    s  = qa @ kb^T                      [la, lb]      (pad k-cols are exactly 0)
    p  = exp(s)                                       (pad cols: exp(0) = 1.0)
    den[q] = sum_k p[q, k] - n_pad                    (exact pad correction)
    g[q] = valid(q) / (la * den[q])
    w[k] = sum_q g[q] p[q, k]           <- collapses the mean over queries
    emb  = (w^T B) @ Wv + bv            <- collapses attn@V and the V projection

Distribution: 64 pairs -> 8 slots x 8 cores (one shared SPMD program, shapes
fixed per slot to the max over cores; pairs bin-packed by length so padding is
small).

Perf notes vs the first version:
  - A/B uploaded BOTH pre-transposed (DIM-major, fp8e4m3, DIM zero-padded to
    768) for the Q/K path AND natural-layout bf16 for the value path; no
    on-device transposes at all.
  - Projections and QK^T run as fp8 DoubleRow matmuls (2 contraction rows per
    partition, 0.5 cyc/row).  INNER=256 = 2x128 maps exactly onto the
    DoubleRow pair dim for the scores.
  - Value path (w^T B, Wv^T u) in bf16 (1 cyc/row, no small-N penalty).
  - exp() is one activation per q-tile over a [128, plk] PSUM span with a
    single accumulator read for den.
"""

import os
import sys

sys.path.insert(0, "/opt/trn_rl_repo")

import numpy as np

B, LA, LB, DIM, INNER, OUTER = 64, 1024, 1024, 640, 256, 1024
NCORES, NSLOTS, P = 8, 8, 128
SCALE = 1.0 / np.sqrt(INNER)
DT = DIM // P       # 5 d-chunks of 128
DJ = 3              # DoubleRow d-pair chunks (768 = 3 * 256)
DPAD = DJ * 2 * P   # 768

LAST_EXEC_TIME_NS = None


def _chunks(total, step=512):
    out, off = [], 0
    while off < total:
        c = min(step, total - off)
        out.append((off, c))
        off += c
    return out


def _plan(la_all, lb_all):
    """Assign pairs to (slot, core); returns swap flags, groups, slot tile shapes."""
    la = np.asarray(la_all, np.int64)
    lb = np.asarray(lb_all, np.int64)
    swap = lb > la
    qa = np.where(swap, lb, la)  # kernel A-side length (>= B-side)
    qb = np.where(swap, la, lb)
    at = -(-qa // P)
    bt = -(-qb // P)
    order = np.argsort(-(at * 1024 + bt), kind="stable")
    groups = [list(order[s * NCORES:(s + 1) * NCORES]) for s in range(NSLOTS)]
    C1, C2 = 2000.0, 200.0

    def gcost(g):
        ma = max(at[i] for i in g)
        mb = max(bt[i] for i in g)
        return C1 * (ma + mb) + C2 * ma * mb

    rng = np.random.default_rng(0)
    cost = [gcost(g) for g in groups]
    s1s = rng.integers(0, NSLOTS, 30000)
    s2s = rng.integers(0, NSLOTS, 30000)
    i1s = rng.integers(0, NCORES, 30000)
    i2s = rng.integers(0, NCORES, 30000)
    for s1, s2, i1, i2 in zip(s1s, s2s, i1s, i2s):
        if s1 == s2:
            continue
        g1 = groups[s1][:]
        g2 = groups[s2][:]
        g1[i1], g2[i2] = groups[s2][i2], groups[s1][i1]
        n1, n2 = gcost(g1), gcost(g2)
        if n1 + n2 < cost[s1] + cost[s2] - 1e-9:
            groups[s1], groups[s2] = g1, g2
            cost[s1], cost[s2] = n1, n2
    slot_at = [int(max(at[i] for i in g)) for g in groups]
    slot_bt = [int(max(bt[i] for i in g)) for g in groups]
    return swap, qa, qb, groups, slot_at, slot_bt


def _build_program(slot_at, slot_bt):
    import concourse.bass as bass  # noqa: F401
    import concourse.mybir as mybir
    import concourse.tile as tile
    from concourse import bacc

    F32 = mybir.dt.float32
    F32R = mybir.dt.float32r
    BF16 = mybir.dt.bfloat16
    FP8 = mybir.dt.float8e4
    Exp = mybir.ActivationFunctionType.Exp
    Ident = mybir.ActivationFunctionType.Identity
    DR = mybir.MatmulPerfMode.DoubleRow
    Alu = mybir.AluOpType

    tot_at = sum(slot_at)
    tot_bt = sum(slot_bt)
    cum_at = np.concatenate([[0], np.cumsum(slot_at)]).astype(int)
    cum_bt = np.concatenate([[0], np.cumsum(slot_bt)]).astype(int)

    nc = bacc.Bacc("TRN2", target_bir_lowering=False, debug=False,
                   num_devices=NCORES)

    # transposed fp8 inputs: [p, j, i, tok] = X[tok, j*256 + i*128 + p]
    at8_d = nc.dram_tensor("at8", [P, DJ, 2, tot_at * P], FP8,
                           kind="ExternalInput")
    bt8_d = nc.dram_tensor("bt8", [P, DJ, 2, tot_bt * P], FP8,
                           kind="ExternalInput")
    # natural bf16 inputs: [p, T, d] = X[T*128 + p, d]
    an_d = nc.dram_tensor("an16", [P, tot_at, DIM], BF16, kind="ExternalInput")
    bn_d = nc.dram_tensor("bn16", [P, tot_bt, DIM], BF16, kind="ExternalInput")
    gs_a_d = nc.dram_tensor("gs_a", [P, tot_at], F32, kind="ExternalInput")
    gs_b_d = nc.dram_tensor("gs_b", [P, tot_bt], F32, kind="ExternalInput")
    npa_d = nc.dram_tensor("npa", [P, NSLOTS], F32, kind="ExternalInput")
    npb_d = nc.dram_tensor("npb", [P, NSLOTS], F32, kind="ExternalInput")
    wq_d = nc.dram_tensor("wq8", [P, DJ, 2, INNER], FP8, kind="ExternalInput")
    wk_d = nc.dram_tensor("wk8", [P, DJ, 2, INNER], FP8, kind="ExternalInput")
    wv_d = nc.dram_tensor("wv16", [P, DT, OUTER], BF16, kind="ExternalInput")
    bv_d = nc.dram_tensor("bv", [P, OUTER // P], F32, kind="ExternalInput")
    idr_d = nc.dram_tensor("idr", [P, P], F32R, kind="ExternalInput")
    idb_d = nc.dram_tensor("idb", [P, P], BF16, kind="ExternalInput")
    emb_d = nc.dram_tensor("emb", [P, OUTER // P, 2 * NSLOTS], F32,
                           kind="ExternalOutput")

    with tile.TileContext(nc) as tc:
        with (
            tc.tile_pool(name="const", bufs=1) as cpool,
            tc.tile_pool(name="ain", bufs=3) as apool,
            tc.tile_pool(name="proj", bufs=2) as ppool,
            tc.tile_pool(name="pexp", bufs=12) as epool,
            tc.tile_pool(name="small", bufs=7) as spool,
            tc.tile_pool(name="late", bufs=2) as lpool,
            tc.tile_pool(name="psA", bufs=3, space="PSUM") as psA,
            tc.tile_pool(name="psW", bufs=1, space="PSUM") as psW,
        ):
            # ---- constants (DMA-ordered: slot-0 critical path first) ----
            wq_sb = cpool.tile([P, DJ, 2, INNER], FP8, tag="wq")
            wk_sb = cpool.tile([P, DJ, 2, INNER], FP8, tag="wk")
            bv_sb = cpool.tile([P, OUTER // P], F32, tag="bv")
            idr_sb = cpool.tile([P, P], F32R, tag="idr")
            npa_sb = cpool.tile([P, NSLOTS], F32, tag="npa")
            npb_sb = cpool.tile([P, NSLOTS], F32, tag="npb")
            gs_a_sb = cpool.tile([P, tot_at], F32, tag="gsa")
            gs_b_sb = cpool.tile([P, tot_bt], F32, tag="gsb")
            wv_sb = cpool.tile([P, DT, OUTER], BF16, tag="wv")
            urows_sb = cpool.tile([2 * NSLOTS, DIM], F32R, tag="urows")
            for sb, d in ((wq_sb, wq_d), (wk_sb, wk_d)):
                nc.sync.dma_start(sb[:], d[:])

            inbufs = {}

            def load_slot(s, qk_only=False, nat_only=False):
                at_s, bt_s = int(slot_at[s]), int(slot_bt[s])
                if not nat_only:
                    a8 = apool.tile([P, DJ, 2, at_s * P], FP8, tag="a8")
                    b8 = apool.tile([P, DJ, 2, bt_s * P], FP8, tag="b8")
                    nc.sync.dma_start(
                        a8[:],
                        at8_d[:, :, :, cum_at[s] * P:(cum_at[s] + at_s) * P])
                    nc.sync.dma_start(
                        b8[:],
                        bt8_d[:, :, :, cum_bt[s] * P:(cum_bt[s] + bt_s) * P])
                    inbufs[s] = (a8, b8, None, None)
                if not qk_only:
                    an = apool.tile([P, at_s, DIM], BF16, tag="an")
                    bn = apool.tile([P, bt_s, DIM], BF16, tag="bn")
                    nc.sync.dma_start(an[:],
                                      an_d[:, cum_at[s]:cum_at[s] + at_s, :])
                    nc.sync.dma_start(bn[:],
                                      bn_d[:, cum_bt[s]:cum_bt[s] + bt_s, :])
                    a8, b8, _, _ = inbufs[s]
                    inbufs[s] = (a8, b8, an, bn)

            projbufs = {}

            def proj_gen(s):
                """fp8 DoubleRow projections: qT/kT [p, m, tok]."""
                at_s, bt_s = int(slot_at[s]), int(slot_bt[s])
                pla, plb = at_s * P, bt_s * P
                a8, b8, an, bn = inbufs.pop(s)
                qaT = ppool.tile([P, 2, pla], FP8, tag="qaT")
                kaT = ppool.tile([P, 2, pla], FP8, tag="kaT")
                qbT = ppool.tile([P, 2, plb], FP8, tag="qbT")
                kbT = ppool.tile([P, 2, plb], FP8, tag="kbT")
                projbufs[s] = (qaT, kaT, qbT, kbT, an, bn)
                for src, pl, dst, w_sb in (
                        (a8, pla, qaT, wq_sb),
                        (a8, pla, kaT, wk_sb),
                        (b8, plb, qbT, wq_sb),
                        (b8, plb, kbT, wk_sb)):
                    for m in range(2):
                        pp = psA.tile([P, 1024], F32, tag="mm")
                        for j in range(DJ):
                            for co, cl in _chunks(pl):
                                nc.tensor.matmul(
                                    pp[:, co:co + cl],
                                    w_sb[:, j, :, m * P:(m + 1) * P],
                                    src[:, j, :, co:co + cl],
                                    start=(j == 0), stop=(j == DJ - 1),
                                    perf_mode=DR)
                        # plain fp8 cast: the q bias rides the ones-row of
                        # A/Wq, the softmax scale rides the exp activation,
                        # and k needs no bias (softmax shift-invariance)
                        nc.vector.tensor_copy(dst[:, m, :], pp[:, :pl])
                        yield

            def tail_gen(s, dr, wr, plk, nk, knat):
                """Deferred per-direction epilogue: transpose w, compute u.

                First step (run eagerly at direction end): wrow copy, freeing
                the wr psum slot.  Later steps are drained one per q-tile
                of the following direction so the PE never idles on the
                wrow/wcol dependency chain.
                """
                wrow = lpool.tile([1, 1024], F32R, tag="wrow")
                if dr == 0:
                    nc.scalar.copy(wrow[0:1, :plk], wr[0:1, :plk])
                else:
                    nc.vector.tensor_copy(wrow[0:1, :plk], wr[0:1, :plk])
                yield
                wt = psA.tile([P, 1024], F32, tag="mm")
                for kt in range(nk):
                    nc.tensor.matmul(
                        wt[:, 2 * kt:2 * kt + 2],
                        wrow[0:1, kt * P:(kt + 1) * P],
                        idr_sb[0:1, 0:2], start=True, stop=True)
                wcol = spool.tile([P, 8], BF16, tag="wcol")
                nc.vector.tensor_copy(
                    wcol[:, :nk],
                    wt[:, :2 * nk].rearrange(
                        "p (k two) -> p k two", two=2)[:, :, 0])
                yield
                # u row = w^T @ Knat   (bf16); ur reuses the wr psum slot
                ur = psW.tile([1, 1024], F32, tag="wr")
                for co, cl in _chunks(DIM):
                    for kt in range(nk):
                        nc.tensor.matmul(
                            ur[0:1, co:co + cl],
                            wcol[:, kt:kt + 1],
                            knat[:, kt, co:co + cl],
                            start=(kt == 0), stop=(kt == nk - 1))
                ursb = lpool.tile([1, DIM], F32R, tag="ursb")
                if dr == 0:
                    nc.vector.tensor_copy(ursb[:], ur[0:1, :DIM])
                else:
                    nc.scalar.copy(ursb[:], ur[0:1, :DIM])
                nc.sync.dma_start(urows_sb[2 * s + dr:2 * s + dr + 1, :],
                                  ursb[:])

            # unified deferred-work queue: wacc pairs and direction tails are
            # issued ~2 q-tiles late (crossing direction/slot boundaries) so
            # the scalar->vector dependency chain never stalls the in-order
            # PE queue
            work = []

            def drain_work(slack):
                # Pops exhausted items freely; steps the head generator, but
                # keeps stepping whenever the backlog exceeds the hard bound
                # so ring-buffer reuse distances stay within the pool sizes.
                while len(work) > slack:
                    gen = work[0]
                    if next(gen, StopIteration) is StopIteration:
                        work.pop(0)
                        continue
                    if slack and len(work) <= 4:
                        break

            def attn_gen(s):
                at_s, bt_s = int(slot_at[s]), int(slot_bt[s])
                qaT, kaT, qbT, kbT, an, bn = projbufs.pop(s)
                for dr in range(2):
                    if dr == 0:  # A queries over B keys
                        QT, KT, nq, nk = qaT, kbT, at_s, bt_s
                        g_sb, g_off = gs_a_sb, cum_at[s]
                        np_sb = npb_sb
                        knat = bn
                    else:
                        QT, KT, nq, nk = qbT, kaT, bt_s, at_s
                        g_sb, g_off = gs_b_sb, cum_bt[s]
                        np_sb = npa_sb
                        knat = an
                    plk = nk * P
                    kch = _chunks(plk)
                    wr = psW.tile([1, 1024], F32, tag="wr")
                    den2 = None

                    def wacc_gen(q0, qn, gcol2, ptiles, wr=wr, kch=kch,
                                 nq=nq):
                        for qp in range(q0, qn + 1):
                            pt = ptiles[qp]
                            for co, cl in kch:
                                nc.tensor.matmul(
                                    wr[0:1, co:co + cl],
                                    gcol2[:, qp - q0:qp - q0 + 1],
                                    pt[:, co:co + cl],
                                    start=(qp == 0), stop=(qp == nq - 1))
                        return
                        yield

                    p_tiles = {}
                    for qt in range(nq):
                        sc = psA.tile([P, 1024], F32, tag="mm")
                        for co, cl in kch:
                            nc.tensor.matmul(
                                sc[:, co:co + cl],
                                QT[:, :, qt * P:(qt + 1) * P],
                                KT[:, :, co:co + cl],
                                start=True, stop=True, perf_mode=DR)
                        if qt % 2 == 0:
                            den2 = spool.tile([P, 2], F32, tag="den")
                        p_sb = epool.tile([P, 1024], BF16, tag="p")
                        p_tiles[qt] = p_sb
                        # p = exp(s / sqrt(INNER)); the softmax scale rides
                        # the activation, the q bias rides the ones-row of A
                        nc.scalar.activation(
                            p_sb[:, :plk], sc[:, :plk], Exp, scale=SCALE,
                            accum_out=den2[:, qt % 2:qt % 2 + 1])
                        if qt % 2 == 1 or qt == nq - 1:
                            q0 = qt - (qt % 2)
                            npair = qt - q0 + 1
                            dpair = den2[:, :npair]
                            # den -= pad count (pad cols are exactly exp(0)=1)
                            nc.vector.tensor_scalar_sub(
                                dpair, dpair, np_sb[:, s:s + 1])
                            rec2 = spool.tile([P, 2], F32, tag="rec")
                            nc.vector.reciprocal(rec2[:, :npair], dpair)
                            gcol2 = spool.tile([P, 2], BF16, tag="gc")
                            nc.vector.tensor_tensor(
                                gcol2[:, :npair], rec2[:, :npair],
                                g_sb[:, g_off + q0:g_off + q0 + npair],
                                Alu.mult)
                            work.append(wacc_gen(q0, qt, gcol2, p_tiles))
                            p_tiles = {}
                        drain_work(2)
                        yield
                    work.append(tail_gen(s, dr, wr, plk, nk, knat))
                    yield

            # software pipeline: slot s attention interleaved with slot s+1
            # projections; input DMA prefetched ~two slots ahead; DMA queue
            # ordered so slot-0 projections can start ASAP
            load_slot(0, qk_only=True)
            for sb, d in ((gs_a_sb, gs_a_d), (gs_b_sb, gs_b_d),
                          (npa_sb, npa_d), (npb_sb, npb_d),
                          (idr_sb, idr_d)):
                nc.sync.dma_start(sb[:], d[:])
            load_slot(0, nat_only=True)
            load_slot(1)
            nc.sync.dma_start(bv_sb[:], bv_d[:])
            nc.sync.dma_start(wv_sb[:], wv_d[:])
            for _ in proj_gen(0):
                pass
            for s in range(NSLOTS):
                if s + 2 < NSLOTS:
                    load_slot(s + 2)
                ag = attn_gen(s)
                pg = proj_gen(s + 1) if s + 1 < NSLOTS else None
                for _ in ag:
                    if pg is not None:
                        if next(pg, StopIteration) is StopIteration:
                            pg = None
                if pg is not None:
                    for _ in pg:
                        pass
            drain_work(0)

            # ---- final: E = Wv^T U + bv ----
            u_sb = cpool.tile([P, DT, 2 * NSLOTS], BF16, tag="usb")
            for dt in range(DT):
                ut = psA.tile([P, 1024], F32, tag="mm")
                nc.tensor.matmul(
                    ut[:, :2 * NSLOTS],
                    urows_sb[:, dt * P:(dt + 1) * P],
                    idr_sb[0:2 * NSLOTS, 0:2 * NSLOTS],
                    start=True, stop=True)
                nc.vector.tensor_copy(u_sb[:, dt, :], ut[:, :2 * NSLOTS])
            e_sb = cpool.tile([P, OUTER // P, 2 * NSLOTS], F32, tag="esb")
            for oc in range(OUTER // P):
                ep = psA.tile([P, 1024], F32, tag="mm")
                for dt in range(DT):
                    nc.tensor.matmul(
                        ep[:, :2 * NSLOTS],
                        wv_sb[:, dt, oc * P:(oc + 1) * P],
                        u_sb[:, dt, :],
                        start=(dt == 0), stop=(dt == DT - 1))
                nc.vector.tensor_scalar_add(e_sb[:, oc, :], ep[:, :2 * NSLOTS],
                                            bv_sb[:, oc, None])
            nc.sync.dma_start(emb_d[:], e_sb[:])

    nc.compile()
    return nc


def _install_profhook():
    import contextlib
    import ctypes
    import types

    import antenv

    if not hasattr(antenv, "axon_hooks"):
        mod = types.ModuleType("antenv.axon_hooks")
        mod._hook = None

        def _set(h):
            mod._hook = h

        def _get():
            return mod._hook

        mod.set_axon_ntff_profile_hook = _set
        mod.get_axon_ntff_profile_hook = _get
        sys.modules["antenv.axon_hooks"] = mod
        antenv.axon_hooks = mod
    from antenv.axon_hooks import set_axon_ntff_profile_hook
    so_path = "/opt/axon/libaxon_pjrt.so"
    if not os.path.exists(so_path):
        return False
    lib = ctypes.CDLL(so_path)
    if not hasattr(lib, "axon_start_nrt_profile"):
        return False
    lib.axon_start_nrt_profile.argtypes = [ctypes.POINTER(ctypes.c_int64),
                                           ctypes.c_size_t]
    lib.axon_start_nrt_profile.restype = ctypes.c_int64
    lib.axon_stop_nrt_profile.argtypes = [ctypes.c_char_p]
    lib.axon_stop_nrt_profile.restype = ctypes.c_int64

    @contextlib.contextmanager
    def _hook(output_dir, device_ids):
        import jax

        jax.devices()
        if device_ids:
            ids = (ctypes.c_int64 * len(device_ids))(*device_ids)
            rc = lib.axon_start_nrt_profile(ids, len(device_ids))
        else:
            rc = lib.axon_start_nrt_profile(None, 0)
        if rc != 0:
            raise RuntimeError(f"axon_start_nrt_profile rc={rc}")
        try:
            yield
        finally:
            n = lib.axon_stop_nrt_profile(str(output_dir).encode())
            print(f"profile: {n} file(s) written to {output_dir}",
                  file=sys.stderr)

    set_axon_ntff_profile_hook(_hook)
    return True


def kernel(a_pad, b_pad, len_a, len_b, Wq, bq, Wk, bk, Wv, bv):
    global LAST_EXEC_TIME_NS
    import ml_dtypes
    FP8 = ml_dtypes.float8_e4m3fn
    BF16 = ml_dtypes.bfloat16

    a_pad = np.ascontiguousarray(np.asarray(a_pad, np.float32))
    b_pad = np.ascontiguousarray(np.asarray(b_pad, np.float32))
    len_a = np.asarray(len_a, np.int32)
    len_b = np.asarray(len_b, np.int32)
    Wq = np.asarray(Wq, np.float32)
    Wk = np.asarray(Wk, np.float32)
    Wv = np.asarray(Wv, np.float32)
    bq = np.asarray(bq, np.float32)
    bv = np.asarray(bv, np.float32)

    swap, qa_len, qb_len, groups, slot_at, slot_bt = _plan(len_a, len_b)
    tot_at, tot_bt = sum(slot_at), sum(slot_bt)
    cum_at = np.concatenate([[0], np.cumsum(slot_at)]).astype(int)
    cum_bt = np.concatenate([[0], np.cumsum(slot_bt)]).astype(int)

    # ---- shared (per-core-identical) inputs ----
    def pack_w8(W, brow=None):
        # [640, INNER] -> [128, 3, 2, INNER] with d = j*256 + i*128 + p;
        # row DIM carries the bias (the data carries 1.0 there)
        Wp = np.zeros((DPAD, W.shape[1]), np.float32)
        Wp[:DIM] = W
        if brow is not None:
            Wp[DIM] = brow
        return np.ascontiguousarray(
            Wp.reshape(DJ, 2, P, W.shape[1]).transpose(2, 0, 1, 3)
        ).astype(FP8)

    wq8 = pack_w8(Wq, bq)
    wk8 = pack_w8(Wk)
    wv16 = np.ascontiguousarray(
        Wv.reshape(DT, P, OUTER).transpose(1, 0, 2)).astype(BF16)
    bv_h = bv.reshape(OUTER // P, P).T.copy()
    idr_h = np.eye(P, dtype=np.float32)
    idb_h = np.eye(P, dtype=np.float32).astype(BF16)

    # ---- per-core inputs ----
    in_maps = []
    for c in range(NCORES):
        abuf = np.zeros((tot_at * P, DPAD), np.float32)
        bbuf = np.zeros((tot_bt * P, DPAD), np.float32)
        abuf[:, DIM] = 1.0
        bbuf[:, DIM] = 1.0
        gs_a = np.zeros((P, tot_at), np.float32)
        gs_b = np.zeros((P, tot_bt), np.float32)
        npa = np.zeros((P, NSLOTS), np.float32)
        npb = np.zeros((P, NSLOTS), np.float32)
        for s in range(NSLOTS):
            i = groups[s][c]
            la_i, lb_i = int(qa_len[i]), int(qb_len[i])
            A = b_pad[i] if swap[i] else a_pad[i]
            Bm = a_pad[i] if swap[i] else b_pad[i]
            abuf[cum_at[s] * P:cum_at[s] * P + la_i, :DIM] = A[:la_i]
            bbuf[cum_bt[s] * P:cum_bt[s] * P + lb_i, :DIM] = Bm[:lb_i]
            ga = np.zeros(slot_at[s] * P, np.float32)
            ga[:la_i] = 1.0 / la_i
            gs_a[:, cum_at[s]:cum_at[s] + slot_at[s]] = \
                ga.reshape(slot_at[s], P).T
            gb = np.zeros(slot_bt[s] * P, np.float32)
            gb[:lb_i] = 1.0 / lb_i
            gs_b[:, cum_bt[s]:cum_bt[s] + slot_bt[s]] = \
                gb.reshape(slot_bt[s], P).T
            npa[:, s] = slot_at[s] * P - la_i
            npb[:, s] = slot_bt[s] * P - lb_i
        # transposed fp8: [tok, 768] -> [128, 3, 2, tok]
        at8 = np.ascontiguousarray(
            abuf.reshape(tot_at * P, DJ, 2, P).transpose(3, 1, 2, 0)
        ).astype(FP8)
        bt8 = np.ascontiguousarray(
            bbuf.reshape(tot_bt * P, DJ, 2, P).transpose(3, 1, 2, 0)
        ).astype(FP8)
        # natural bf16: [tok, 640] -> [128, T, 640]
        an16 = np.ascontiguousarray(
            abuf[:, :DIM].reshape(tot_at, P, DIM).transpose(1, 0, 2)
        ).astype(BF16)
        bn16 = np.ascontiguousarray(
            bbuf[:, :DIM].reshape(tot_bt, P, DIM).transpose(1, 0, 2)
        ).astype(BF16)
        in_maps.append({
            "at8": at8, "bt8": bt8, "an16": an16, "bn16": bn16,
            "gs_a": gs_a, "gs_b": gs_b, "npa": npa, "npb": npb,
            "wq8": wq8, "wk8": wk8, "wv16": wv16,
            "bv": bv_h, "idr": idr_h, "idb": idb_h,
        })

    nc = _build_program(slot_at, slot_bt)

    from concourse.bass_utils import run_bass_kernel_spmd

    trace = os.environ.get("BASS_KERNEL_TRACE", "0") == "1"
    if trace:
        _install_profhook()
    res = run_bass_kernel_spmd(nc, in_maps, list(range(NCORES)), trace=trace)
    LAST_EXEC_TIME_NS = res.exec_time_ns

    emb_a = np.zeros((B, OUTER), np.float32)
    emb_b = np.zeros((B, OUTER), np.float32)
    for c in range(NCORES):
        e = res.results[c]["emb"].transpose(1, 0, 2).reshape(OUTER,
                                                            2 * NSLOTS)
        for s in range(NSLOTS):
            i = groups[s][c]
            ea, eb = e[:, 2 * s], e[:, 2 * s + 1]  # A-queries, B-queries
            if swap[i]:
                emb_a[i], emb_b[i] = eb, ea
            else:
                emb_a[i], emb_b[i] = ea, eb
    return emb_a, emb_b
